# revision 33
# baseline (speedup 1.0000x reference)
"""Trainium2 Bass kernel for nn_EConly_85469849190489 (dual-branch molecular
transformer). Data-parallel over batch: 8 NeuronCores x 4 molecules each.

v2 layout strategy (per core):
  - softmax(dist+neg) precomputed on HOST (layer-invariant) -> ednT upload;
    the dist branch needs no device-side row-sum machinery
  - residual stream x kept tokens-major fp32 (LayerNorm via bn_stats);
    feature-major fp16 copies (xT) via PE transpose for matmul moving use
  - attention transposed: scores_T(k,q), pad mask as per-partition ACT bias,
    exp without max pass, A_raw^T feeds A@V with no transposes of A
  - softmax denominators via masked-ones matmuls packed into 32-row strips
    (per-strip start= resets, no PSUM memsets)
  - rsqrt via exp(-0.5*ln(x)) so the ACT engine never leaves the exp table
  - FFN2 computed tokens-major with hT as stationary (no ff2T transpose)
  - step-major emission skewed across molecules to keep engines busy
"""

import numpy as np
from contextlib import ExitStack

import concourse.bass as bass
import concourse.tile as tile
from concourse import mybir
from concourse.tile import TileContext, ScopedClock

F16 = mybir.dt.float16
F32 = mybir.dt.float32
F8 = mybir.dt.float8e4
DR = mybir.MatmulPerfMode.DoubleRow
AF = mybir.ActivationFunctionType
ALU = mybir.AluOpType

WS = 64.0  # host-side fp8 weight scale
QS = 16.0  # osc (attention output) fp8 scale

# scheduling/engine-placement knobs (tuned via TimelineSim)
CONFIG = {
    "ps_split": True,   # True: po/pr get their own PSUM pool (ps_o bufs=2)
    "ps_s_bufs": 2,     # score-tile double buffering (2 banks each)
    "ps_a_bufs": 2,     # general PSUM pool depth (1 bank each)
    "skew": (0, 2, 3),  # emission offsets of (proj, attn, ffn) stages
    "aw_pool": 0,       # every Nth Aw mul on gpsimd/Pool (0 = never)
    "ln_pool": False,   # LN applies on Pool
    "exp4": False,      # single [128,4,512] score tile + one 4-head exp
    "ps_o_bufs": 2,     # attention po/pr tile pairs in flight
    "fp8_qkv": True,    # xT + q/k/v weights in fp8 (DoubleRow projections)
    "fp8_wo": True,     # osc + W_o in fp8 (DoubleRow)
    "fp8_ffn": "ffn1",  # fp8 only in FFN1 (FFN2 in fp8 breaks accuracy)
}

B, S_FULL, F_IN, D, H, L, DFF, C = 32, 512, 64, 256, 8, 4, 1024, 4
U1, U2 = 512, 256
NH, DEPTH, DH = 4, 32, 128
NEG = -1.0e9
EPS_LN = 1.0e-6
NCORES = 8
MPC = 4  # molecules per core
P = 128

_PATCHED = False


def _patch_drain():
    """This walrus build allows only one sync-wait per CTRL instruction; the
    TileContext exit drain carries one wait per live semaphore. Spread the
    extras across single-wait SP nops."""
    global _PATCHED
    if _PATCHED:
        return

    def _drain_and_barrier(self, tick_clock, wait_clock):
        nc = self.nc
        drain_inst = nc.sync.drain()
        wait_clock.add_sem_waits(
            drain_inst.ins, ScopedClock({None: tick_clock.global_clock})
        )
        si = drain_inst.ins.sync_info
        waits = list(si.on_wait) if si is not None else []
        if len(waits) > 1:
            si.on_wait = waits[:1]
            drain_inst.ins.sync_info = si
            for w in waits[1:]:
                nop = nc.sync.nop(nofuse=True)
                nop.ins.sync_info = mybir.SyncInfo(on_wait=[w], on_update=[])
        nc.all_engine_barrier()
        popped = nc._tile_sem_poison_stack.pop()
        assert popped is self._sem_poison
        nc.clear_and_free_semaphores(list(self.sems.allocated().values()))
        nc.all_engine_barrier()

    TileContext._drain_and_barrier = _drain_and_barrier
    _PATCHED = True


def _split_multiwaits(nc):
    """This walrus build allows only one sync-wait per instruction: move extra
    waits onto same-engine nops placed immediately before the instruction."""
    n = 0
    for fn in nc.m.functions:
        for blk in fn.blocks:
            newl = []
            for inst in blk.instructions:
                si = inst.sync_info
                if si is not None and len(si.on_wait) > 1:
                    waits = list(si.on_wait)
                    for i, w in enumerate(waits[:-1]):
                        nop = mybir.InstNoOp(
                            name=f"{inst.name}-w{i}", ins=[], outs=[], engine=inst.engine
                        )
                        nop.sync_info = mybir.SyncInfo(on_wait=[w], on_update=[])
                        newl.append(nop)
                        n += 1
                    si.on_wait = waits[-1:]
                    inst.sync_info = si
                newl.append(inst)
            blk.instructions = newl
    return n


def _chunks(S):
    """[(c, p0, pc)] partition chunks covering S: offset p0, size pc<=128."""
    out = []
    c = 0
    while c * P < S:
        out.append((c, c * P, min(P, S - c * P)))
        c += 1
    return out


def build_program(slot_S, L_run=L, taps=(), time_loop=0):
    """Build the SPMD per-core program. slot_S: list of MPC sequence lengths
    (each a multiple of 8, <= 512). taps: debug tensor names to stream out."""
    _patch_drain()
    nc = bass.Bass(trn_type="TRN2", detect_race_conditions=False)

    dram = {}
    taps = set(taps)

    def tap(name, ap, shape=None):
        if name not in taps:
            return
        shape = list(shape if shape is not None else ap.shape)
        t = nc.dram_tensor("dbg_" + name, shape, ap.dtype, kind="ExternalOutput")
        nc.sync.dma_start(out=t[...], in_=ap)

    def din(name, shape, dt):
        dram[name] = nc.dram_tensor(name, list(shape), dt, kind="ExternalInput")
        return dram[name]

    for m, S in enumerate(slot_S):
        KC = len(_chunks(S))
        din(f"mft{m}", (64, S), F16)
        din(f"adjT{m}", (S, S), F16)
        din(f"ednT{m}", (S, S), F16)
        din(f"negc{m}", (KC, 128, 1), F32)
        din(f"poolm{m}", (KC, 128, 1), F16)
    din("wembed", (64, D), F16)
    QKV_DT = F8 if CONFIG["fp8_qkv"] else F16
    WO_DT = F8 if CONFIG["fp8_wo"] else F16
    ffn_mode = CONFIG["fp8_ffn"]
    FFN1_ON = ffn_mode in (True, "ffn1")
    FFN2_ON = ffn_mode in (True, "ffn2")
    FFN1_DT = F8 if FFN1_ON else F16
    FFN2_DT = F8 if FFN2_ON else F16
    din("wq", (L, 2, D, DH), QKV_DT)
    din("wk", (L, 2, D, DH), QKV_DT)
    din("wv", (L, D, 2 * DH), QKV_DT)
    din("wo", (L, D, D), WO_DT)
    din("wf1", (L, D, DFF), FFN1_DT)
    din("wf2", (L, DFF, D), FFN2_DT)
    din("wh1", (C, D, U1), F16)
    din("wh2", (C, U1, U2), F16)
    din("chain", (C, MPC, U2), F32)
    din("ident32", (128, 128), F32)
    din("ident16", (128, 128), F16)
    out_dram = nc.dram_tensor("out", [C, MPC, U2], F32, kind="ExternalOutput")

    MOLS = list(range(len(slot_S)))

    with TileContext(nc) as tc:
        with ExitStack() as ctx:
            pers = ctx.enter_context(tc.tile_pool(name="pers", bufs=1))
            wpool = ctx.enter_context(tc.tile_pool(name="wpool", bufs=2))
            wkq = ctx.enter_context(tc.tile_pool(name="wkq", bufs=1))
            wk2 = ctx.enter_context(tc.tile_pool(name="work", bufs=CONFIG.get("wk2_bufs", 2)))
            wk3 = ctx.enter_context(tc.tile_pool(name="work3", bufs=CONFIG.get("wk3_bufs", 6)))
            # PSUM budget is 8 banks total
            ps_s = ctx.enter_context(
                tc.tile_pool(name="ps_s", bufs=CONFIG["ps_s_bufs"], space="PSUM")
            )
            if CONFIG["ps_split"]:
                ps_o = ctx.enter_context(
                    tc.tile_pool(name="ps_o", bufs=CONFIG["ps_o_bufs"], space="PSUM")
                )
            ps_a = ctx.enter_context(
                tc.tile_pool(name="ps_a", bufs=CONFIG["ps_a_bufs"], space="PSUM")
            )
            if not CONFIG["ps_split"]:
                ps_o = ps_a

            # ---- persistent constants / state ----
            ident_32 = pers.tile([128, 128], F32, tag="id32")
            nc.sync.dma_start(out=ident_32, in_=dram["ident32"][:, :])
            ident_16 = pers.tile([128, 128], F16, tag="id16")
            nc.sync.dma_start(out=ident_16, in_=dram["ident16"][:, :])
            ones16 = pers.tile([128, 32], F16, tag="ones16")
            nc.vector.memset(ones16, 1.0)
            eps_col = pers.tile([128, 1], F32, tag="eps")
            nc.vector.memset(eps_col, EPS_LN)
            eps12_col = pers.tile([128, 1], F32, tag="eps12")
            nc.vector.memset(eps12_col, 1.0e-12)
            chain_sb = pers.tile([C, MPC * U2], F32, tag="chain")
            nc.sync.dma_start(
                out=chain_sb, in_=dram["chain"].rearrange("k m u -> k (m u)")
            )

            negc, poolm, xT, xtok, adjT, ednT, mft = [], [], [], [], [], [], []
            for m, S in enumerate(slot_S):
                ch = _chunks(S)
                KC = len(ch)
                t = pers.tile([128, KC, 1], F32, tag=f"negc{m}", name=f"negc{m}")
                nc.sync.dma_start(
                    out=t, in_=dram[f"negc{m}"].rearrange("c p one -> p c one")
                )
                negc.append(t)
                t = pers.tile([128, KC, 1], F16, tag=f"poolm{m}", name=f"poolm{m}")
                nc.sync.dma_start(
                    out=t, in_=dram[f"poolm{m}"].rearrange("c p one -> p c one")
                )
                poolm.append(t)
                xT.append(pers.tile([128, 2, 512], QKV_DT, tag=f"xT{m}", name=f"xT{m}"))
                xtok.append(pers.tile([128, KC, D], F16, tag=f"xtok{m}", name=f"xtok{m}"))
                t = pers.tile([128, KC, S], F16, tag=f"adjT{m}", name=f"adjT{m}")
                for c, p0, pc in ch:
                    nc.sync.dma_start(
                        out=t[0:pc, c, :], in_=dram[f"adjT{m}"][p0 : p0 + pc, :]
                    )
                adjT.append(t)
                t = pers.tile([128, KC, S], F16, tag=f"ednT{m}", name=f"ednT{m}")
                for c, p0, pc in ch:
                    nc.sync.dma_start(
                        out=t[0:pc, c, :], in_=dram[f"ednT{m}"][p0 : p0 + pc, :]
                    )
                ednT.append(t)
                t = pers.tile([128, S], F16, tag=f"mft{m}", name=f"mft{m}")
                nc.sync.dma_start(out=t[0:64, :], in_=dram[f"mft{m}"][:, :])
                mft.append(t)

            wh1_sb = pers.tile([128, C, 2, U1], F16, tag="wh1")
            nc.sync.dma_start(
                out=wh1_sb, in_=dram["wh1"].rearrange("k (c p) n -> p k c n", p=128)
            )
            wh2_sb = pers.tile([128, C, 4, U2], F16, tag="wh2")
            nc.sync.dma_start(
                out=wh2_sb, in_=dram["wh2"].rearrange("k (c p) n -> p k c n", p=128)
            )
            wembed_sb = pers.tile([128, D], F16, tag="wembed")
            nc.sync.dma_start(out=wembed_sb[0:64, :], in_=dram["wembed"][:, :])

            prow = pers.tile([128, MPC * D], F32, tag="prow")
            MM = nc.tensor.matmul

            # per-mol tiles that live across layer steps
            qkT = [
                wkq.tile([128, 4, S], F16, tag=f"qkT{m}", name=f"qkT{m}")
                for m, S in enumerate(slot_S)
            ]
            Vt = [
                wkq.tile([128, KC_, 2 * DH], F16, tag=f"Vt{m}", name=f"Vt{m}")
                for m, (S, KC_) in enumerate(
                    (S, len(_chunks(S))) for S in slot_S
                )
            ]
            osc = [
                wkq.tile([128, 2, 512], WO_DT, tag=f"osc{m}", name=f"osc{m}")
                for m, S in enumerate(slot_S)
            ]

            from contextlib import nullcontext

            with (tc.For_i(0, time_loop, 1) if time_loop else nullcontext()):
                # ---- phase 0: embed ----
                for m, S in enumerate(slot_S):
                    ch = _chunks(S)
                    for c2 in range(2):
                        pse = ps_a.tile([128, 512], F32, tag="a")
                        MM(
                            pse[:, 0:S],
                            wembed_sb[0:64, c2 * 128 : (c2 + 1) * 128],
                            mft[m][0:64, :],
                            start=True,
                            stop=True,
                        )
                        nc.scalar.copy(out=xT[m][:, c2, 0:S], in_=pse[:, 0:S])
                    for c, p0, pc in ch:
                        pse = ps_a.tile([128, 512], F32, tag="a")
                        MM(
                            pse[0:pc, 0:D],
                            mft[m][0:64, p0 : p0 + pc],
                            wembed_sb[0:64, :],
                            start=True,
                            stop=True,
                        )
                        nc.vector.tensor_copy(out=xtok[m][0:pc, c, :], in_=pse[0:pc, 0:D])

                tap("xT0", xT[0][:, :, :])
                tap("xtok0", xtok[0][:, :, :])

                # ---- layers ----
                for l in range(L_run):
                    wq_sb = wpool.tile([128, 2, 2, DH], QKV_DT, tag="wq")
                    nc.sync.dma_start(
                        out=wq_sb,
                        in_=dram["wq"][l].rearrange("b (c p) n -> p b c n", p=128),
                    )
                    wk_sb = wpool.tile([128, 2, 2, DH], QKV_DT, tag="wk")
                    nc.sync.dma_start(
                        out=wk_sb,
                        in_=dram["wk"][l].rearrange("b (c p) n -> p b c n", p=128),
                    )
                    wv_sb = wpool.tile([128, 2, 2 * DH], QKV_DT, tag="wv")
                    nc.sync.dma_start(
                        out=wv_sb, in_=dram["wv"][l].rearrange("(c p) n -> p c n", p=128)
                    )
                    wo_sb = wpool.tile([128, 2, D], WO_DT, tag="wo")
                    nc.sync.dma_start(
                        out=wo_sb, in_=dram["wo"][l].rearrange("(c p) n -> p c n", p=128)
                    )
                    wf1_sb = wpool.tile([128, 2, DFF], FFN1_DT, tag="wf1")
                    nc.sync.dma_start(
                        out=wf1_sb, in_=dram["wf1"][l].rearrange("(c p) n -> p c n", p=128)
                    )
                    wf2_sb = wpool.tile([128, 8, D], FFN2_DT, tag="wf2")
                    nc.sync.dma_start(
                        out=wf2_sb, in_=dram["wf2"][l].rearrange("(c p) n -> p c n", p=128)
                    )

                    # ---- S1+S2: q/k/v projections for mol m ----
                    def proj_qkv(m):
                        S = slot_S[m]
                        ch = _chunks(S)
                        for br in range(2):
                            for which, w_sb in ((0, wq_sb), (1, wk_sb)):
                                pp = ps_a.tile([128, 512], F32, tag="a")
                                if CONFIG["fp8_qkv"]:
                                    MM(
                                        pp[:, 0:S],
                                        w_sb[:, br, :, :],
                                        xT[m][:, :, 0:S],
                                        start=True,
                                        stop=True,
                                        perf_mode=DR,
                                    )
                                else:
                                    for cc in range(2):
                                        MM(
                                            pp[:, 0:S],
                                            w_sb[:, br, cc, :],
                                            xT[m][:, cc, 0:S],
                                            start=(cc == 0),
                                            stop=(cc == 1),
                                        )
                                # qkT slots: 0,1 = qT br0/br1; 2,3 = kT br0/br1
                                slot = which * 2 + br
                                qs = (1.0 / WS) if CONFIG["fp8_qkv"] else 1.0
                                if which == 0:
                                    nc.scalar.activation(
                                        qkT[m][:, slot, :],
                                        pp[:, 0:S],
                                        AF.Copy,
                                        scale=qs,
                                    )
                                else:
                                    nc.vector.tensor_scalar(
                                        out=qkT[m][:, slot, :],
                                        in0=pp[:, 0:S],
                                        scalar1=qs,
                                        scalar2=None,
                                        op0=ALU.mult,
                                    )
                        for c, p0, pc in ch:
                            pv = ps_a.tile([128, 512], F32, tag="a")
                            if CONFIG["fp8_qkv"]:
                                MM(
                                    pv[0:pc, 0 : 2 * DH],
                                    xT[m][:, :, p0 : p0 + pc],
                                    wv_sb[:, :, :],
                                    start=True,
                                    stop=True,
                                    perf_mode=DR,
                                )
                            else:
                                for cc in range(2):
                                    MM(
                                        pv[0:pc, 0 : 2 * DH],
                                        xT[m][:, cc, p0 : p0 + pc],
                                        wv_sb[:, cc, :],
                                        start=(cc == 0),
                                        stop=(cc == 1),
                                    )
                            nc.vector.tensor_scalar(
                                out=Vt[m][0:pc, c, :],
                                in0=pv[0:pc, 0 : 2 * DH],
                                scalar1=(1.0 / WS) if CONFIG["fp8_qkv"] else 1.0,
                                scalar2=None,
                                op0=ALU.mult,
                            )

                    # ---- S3: attention both branches for mol m ----
                    def attn(m):
                        S = slot_S[m]
                        ch = _chunks(S)
                        KC = len(ch)
                        for br in range(2):
                            qT = qkT[m][:, 0 + br, :]
                            kT = qkT[m][:, 2 + br, :]
                            MT = ednT[m] if br == 0 else adjT[m]
                            otag = "o" if CONFIG["ps_split"] else "a"
                            # phase 1: scores -> exp -> masked product, per chunk
                            PTs, Aws = [], []
                            for c, p0, pc in ch:
                                PT = wk3.tile([128, 4, S], F16, tag="PT")
                                for pair in range(2):
                                    ps = ps_s.tile([128, 2, 512], F32, tag="s")
                                    for i in range(2):
                                        h = 2 * pair + i
                                        MM(
                                            ps[0:pc, i, 0:S],
                                            kT[32 * h : 32 * h + 32, p0 : p0 + pc],
                                            qT[32 * h : 32 * h + 32, :],
                                            start=True,
                                            stop=True,
                                            tile_position=(32 * h, 0),
                                        )
                                    nc.scalar.activation(
                                        PT[0:pc, 2 * pair : 2 * pair + 2, :],
                                        ps[0:pc, :, 0:S],
                                        AF.Exp,
                                        bias=negc[m][0:pc, c, :],
                                        scale=1.0,
                                    )
                                Aw = wk3.tile([128, 4, S], F16, tag="Aw")
                                nc.vector.tensor_mul(
                                    Aw[0:pc, :, :],
                                    PT[0:pc, :, :],
                                    MT[0:pc, c : c + 1, :].broadcast_to((pc, 4, S)),
                                )
                                PTs.append(PT)
                                Aws.append(Aw)
                            # phase 2: denominator + A@V bursts (short PSUM residency)
                            po = ps_o.tile([128, 512], F32, tag=otag)
                            pr = ps_o.tile([128, 512], F32, tag=otag)
                            for c, p0, pc in ch:
                                for h in range(4):
                                    MM(
                                        pr[32 * h : 32 * h + 32, 0:S],
                                        ones16[0:pc, 0:32],
                                        PTs[c][0:pc, h, :],
                                        start=(c == 0),
                                        stop=(c == KC - 1),
                                        tile_position=(0, 32 * h),
                                        skip_group_check=True,
                                    )
                                    MM(
                                        po[32 * h : 32 * h + 32, 0:S],
                                        Vt[m][
                                            0:pc,
                                            c,
                                            br * DH + 32 * h : br * DH + 32 * h + 32,
                                        ],
                                        Aws[c][0:pc, h, :],
                                        start=(c == 0),
                                        stop=(c == KC - 1),
                                        tile_position=(0, 32 * h),
                                        skip_group_check=True,
                                    )
                            recipb = wk2.tile([128, S], F32, tag="recipb")
                            nc.vector.reciprocal(out=recipb[:, :], in_=pr[:, 0:S])
                            nc.vector.scalar_tensor_tensor(
                                out=osc[m][:, br, 0:S],
                                in0=po[:, 0:S],
                                scalar=QS if CONFIG["fp8_wo"] else 1.0,
                                in1=recipb[:, :],
                                op0=ALU.mult,
                                op1=ALU.mult,
                            )
                        if l == 0 and m == 0:
                            tap("osc0", osc[m][:, :, :])

                    # ---- S4+S5+S6: Wo+LN1, FFN1, FFN2+LN2, next xT for mol m ----
                    def ffn_block(m):
                        S = slot_S[m]
                        ch = _chunks(S)
                        KC = len(ch)
                        # Wo + residual + LN1
                        xres = wk2.tile([128, KC, D], F16, tag="xres")
                        xln1 = wk3.tile([128, KC, D], F16, tag="xln1")
                        mv = wk2.tile([128, KC, 2], F32, tag="mv")
                        if S % 128:
                            pal = ((S % 128) // 32) * 32
                            nc.vector.memset(xres[pal:128, KC - 1, :], 0.0)
                        for c, p0, pc in ch:
                            pw = ps_a.tile([128, 512], F32, tag="a")
                            if CONFIG["fp8_wo"]:
                                MM(
                                    pw[0:pc, 0:D],
                                    osc[m][:, :, p0 : p0 + pc],
                                    wo_sb[:, :, :],
                                    start=True,
                                    stop=True,
                                    perf_mode=DR,
                                )
                            else:
                                for cc in range(2):
                                    MM(
                                        pw[0:pc, 0:D],
                                        osc[m][:, cc, p0 : p0 + pc],
                                        wo_sb[:, cc, :],
                                        start=(cc == 0),
                                        stop=(cc == 1),
                                    )
                            nc.vector.scalar_tensor_tensor(
                                out=xres[0:pc, c, :],
                                in0=pw[0:pc, 0:D],
                                scalar=(1.0 / (WS * QS)) if CONFIG["fp8_wo"] else 1.0,
                                in1=xtok[m][0:pc, c, :],
                                op0=ALU.mult,
                                op1=ALU.add,
                            )
                        bst = wk3.tile([128, KC, 6], F32, tag="bst")
                        for c in range(KC):
                            nc.vector.bn_stats(out=bst[:, c, :], in_=xres[:, c, :])
                            nc.vector.bn_aggr(out=mv[:, c, :], in_=bst[:, c, :])
                        if l == 0 and m == 0:
                            tap("xres0", xres[:, :, :])
                        # rstd = exp(-0.5*ln(var+eps)) - stays in the exp table
                        sd = wk2.tile([128, KC], F32, tag="sd")
                        nc.scalar.activation(
                            sd[:, :], mv[:, :, 1], AF.Ln, bias=eps_col[:, :], scale=1.0
                        )
                        rstd = wk2.tile([128, KC], F32, tag="rstd")
                        nc.scalar.activation(rstd[:, :], sd[:, :], AF.Exp, scale=-0.5)
                        for c, p0, pc in ch:
                            nc.vector.tensor_scalar(
                                out=xln1[0:pc, c, :],
                                in0=xres[0:pc, c, :],
                                scalar1=mv[0:pc, c, 0:1],
                                scalar2=rstd[0:pc, c : c + 1],
                                op0=ALU.subtract,
                                op1=ALU.mult,
                            )
                        if l == 0 and m == 0:
                            tap("xln1_0", xln1[:, :, :])
                        # x_ln1^T (feature-major fp8) via PE transpose (f16 in)
                        xln1T = wk2.tile([128, 2, 512], FFN1_DT, tag="xln1T")
                        for cc in range(2):
                            pt = ps_a.tile([128, 512], F16, tag="a")
                            for c, p0, pc in ch:
                                nc.tensor.transpose(
                                    pt[:, p0 : p0 + pc],
                                    xln1[0:pc, c, cc * 128 : (cc + 1) * 128],
                                    ident_16[0:pc, 0:pc],
                                )
                            nc.scalar.copy(out=xln1T[:, cc, 0:S], in_=pt[:, 0:S])
                        if l == 0 and m == 0:
                            tap("xln1T0", xln1T[:, :, :])
                        # FFN1 (feature-major) with relu -> h^T fp8
                        hT = wk2.tile([128, 8, 512], FFN2_DT, tag="hT")
                        for hc in range(8):
                            pf = ps_a.tile([128, 512], F32, tag="a")
                            if FFN1_ON:
                                MM(
                                    pf[:, 0:S],
                                    wf1_sb[:, :, hc * 128 : (hc + 1) * 128],
                                    xln1T[:, :, 0:S],
                                    start=True,
                                    stop=True,
                                    perf_mode=DR,
                                )
                            else:
                                for cc in range(2):
                                    MM(
                                        pf[:, 0:S],
                                        wf1_sb[:, cc, hc * 128 : (hc + 1) * 128],
                                        xln1T[:, cc, 0:S],
                                        start=(cc == 0),
                                        stop=(cc == 1),
                                    )
                            hs = (1.0 / WS) if FFN1_ON else 1.0
                            if hc % 2 == 0:
                                nc.scalar.activation(
                                    hT[:, hc, 0:S], pf[:, 0:S], AF.Relu, scale=hs
                                )
                            else:
                                nc.vector.tensor_scalar(
                                    out=hT[:, hc, 0:S],
                                    in0=pf[:, 0:S],
                                    scalar1=0.0,
                                    scalar2=hs,
                                    op0=ALU.max,
                                    op1=ALU.mult,
                                )
                        if l == 0 and m == 0:
                            tap("hT0", hT[:, :, :])
                        # FFN2 tokens-major: stationary = hT chunk, moving = wf2
                        mv2 = wk2.tile([128, KC, 2], F32, tag="mv2")
                        xres2 = wk2.tile([128, KC, D], F16, tag="xres2")
                        if S % 128:
                            pal = ((S % 128) // 32) * 32
                            nc.vector.memset(xres2[pal:128, KC - 1, :], 0.0)
                        for c, p0, pc in ch:
                            pf = ps_a.tile([128, 512], F32, tag="a")
                            if FFN2_ON:
                                for j in range(4):
                                    MM(
                                        pf[0:pc, 0:D],
                                        hT[:, 2 * j : 2 * j + 2, p0 : p0 + pc],
                                        wf2_sb[:, 2 * j : 2 * j + 2, :],
                                        start=(j == 0),
                                        stop=(j == 3),
                                        perf_mode=DR,
                                    )
                            else:
                                for hc in range(8):
                                    MM(
                                        pf[0:pc, 0:D],
                                        hT[:, hc, p0 : p0 + pc],
                                        wf2_sb[:, hc, :],
                                        start=(hc == 0),
                                        stop=(hc == 7),
                                    )
                            nc.vector.scalar_tensor_tensor(
                                out=xres2[0:pc, c, :],
                                in0=pf[0:pc, 0:D],
                                scalar=(1.0 / WS) if FFN2_ON else 1.0,
                                in1=xln1[0:pc, c, :],
                                op0=ALU.mult,
                                op1=ALU.add,
                            )
                        bst = wk3.tile([128, KC, 6], F32, tag="bst")
                        for c in range(KC):
                            nc.vector.bn_stats(out=bst[:, c, :], in_=xres2[:, c, :])
                            nc.vector.bn_aggr(out=mv2[:, c, :], in_=bst[:, c, :])
                        sd2 = wk2.tile([128, KC], F32, tag="sd2")
                        nc.scalar.activation(
                            sd2[:, :], mv2[:, :, 1], AF.Ln, bias=eps_col[:, :], scale=1.0
                        )
                        rstd2 = wk2.tile([128, KC], F32, tag="rstd2")
                        nc.scalar.activation(rstd2[:, :], sd2[:, :], AF.Exp, scale=-0.5)
                        for c, p0, pc in ch:
                            nc.vector.tensor_scalar(
                                out=xtok[m][0:pc, c, :],
                                in0=xres2[0:pc, c, :],
                                scalar1=mv2[0:pc, c, 0:1],
                                scalar2=rstd2[0:pc, c : c + 1],
                                op0=ALU.subtract,
                                op1=ALU.mult,
                            )
                        if l == 0 and m == 0:
                            tap("xtok_l0", xtok[m][:, :, :])
                        if l < L_run - 1:
                            for cc in range(2):
                                pt = ps_a.tile([128, 512], F16, tag="a")
                                for c, p0, pc in ch:
                                    nc.tensor.transpose(
                                        pt[:, p0 : p0 + pc],
                                        xtok[m][0:pc, c, cc * 128 : (cc + 1) * 128],
                                        ident_16[0:pc, 0:pc],
                                    )
                                nc.scalar.copy(out=xT[m][:, cc, 0:S], in_=pt[:, 0:S])

                    def pool_mol(m):
                        S = slot_S[m]
                        ch = _chunks(S)
                        pp = ps_a.tile([128, 512], F32, tag="a")
                        for c, p0, pc in ch:
                            MM(
                                pp[0:1, 0:D],
                                poolm[m][0:pc, c, :],
                                xtok[m][0:pc, c, :],
                                start=(c == 0),
                                stop=(c == len(ch) - 1),
                            )
                        nc.vector.tensor_copy(
                            out=prow[0:1, m * D : (m + 1) * D], in_=pp[0:1, 0:D]
                        )

                    # skewed emission: proj runs ahead of attn, attn ahead of
                    # the ffn block, so each engine sees independent work from
                    # neighbouring molecules between dependent stages
                    stages = [proj_qkv, attn, ffn_block]
                    offs = list(CONFIG["skew"])
                    if l == L_run - 1:
                        stages.append(pool_mol)
                        offs.append(offs[-1] + 1)
                    for t in range(len(MOLS) + max(offs)):
                        for si in range(len(stages)):
                            mi = t - offs[si]
                            if 0 <= mi < len(MOLS):
                                stages[si](mi)

                    if l == 0:
                        tap("qkT0", qkT[0][:, :, :])
                        tap("Vt0", Vt[0][:, :, :])

                # ---- contrastive heads (pooling ran as a last-layer stage) ----
                ones32c = pers.tile([128, 1], F32, tag="ones32c")
                nc.vector.memset(ones32c, 1.0)
                # pooled^T (256 x MPC) fp16 via K=1 matmuls off the partition-0 row
                pT_sb = pers.tile([128, 2, MPC], F16, tag="pT")
                ptp = ps_a.tile([128, 512], F32, tag="a")
                for cc in range(2):
                    for m in range(MPC):
                        MM(
                            ptp[:, cc * MPC + m : cc * MPC + m + 1],
                            prow[0:1, m * D + cc * 128 : m * D + (cc + 1) * 128],
                            ones32c[0:1, 0:1],
                            start=(cc == 0 and m == 0),
                            stop=(cc == 1 and m == MPC - 1),
                            skip_group_check=True,
                        )
                nc.vector.tensor_copy(
                    out=pT_sb[:, :, :],
                    in_=ptp[:, 0 : 2 * MPC].rearrange("p (c m) -> p c m", c=2),
                )
                for k in range(C):
                    h1 = wk2.tile([128, 4, MPC], F16, tag="h1")
                    for u in range(4):
                        ph = ps_a.tile([128, 512], F32, tag="a")
                        for cc in range(2):
                            MM(
                                ph[:, 0:MPC],
                                wh1_sb[:, k, cc, u * 128 : (u + 1) * 128],
                                pT_sb[:, cc, :],
                                start=(cc == 0),
                                stop=(cc == 1),
                            )
                        nc.scalar.activation(h1[:, u, :], ph[:, 0:MPC], AF.Relu)
                    h2m = wk2.tile([128, D], F32, tag="h2m")
                    ph2 = ps_a.tile([128, 512], F32, tag="a")
                    for oc in range(2):
                        for uc in range(4):
                            MM(
                                ph2[:, oc * MPC : (oc + 1) * MPC],
                                wh2_sb[:, k, uc, oc * 128 : (oc + 1) * 128],
                                h1[:, uc, :],
                                start=(uc == 0),
                                stop=(uc == 3),
                            )
                    # relu into fp16 tile, transpose to (MPC x 256) rows
                    h2f = wk2.tile([128, 2, MPC], F16, tag="h2f")
                    for oc in range(2):
                        nc.scalar.activation(
                            h2f[:, oc, :], ph2[:, oc * MPC : (oc + 1) * MPC], AF.Relu
                        )
                    pht = ps_a.tile([128, 512], F16, tag="a")
                    for oc in range(2):
                        nc.tensor.transpose(
                            pht[0:MPC, oc * 128 : (oc + 1) * 128],
                            h2f[:, oc, :],
                            ident_16[:, :],
                        )
                    nc.vector.tensor_copy(out=h2m[0:MPC, :], in_=pht[0:MPC, 0:D])
                    # l2 normalize rows: rs = exp(-0.5*ln(ss+1e-12))
                    sq = wk2.tile([128, D], F32, tag="sq")
                    nc.vector.tensor_mul(sq[0:MPC, :], h2m[0:MPC, :], h2m[0:MPC, :])
                    ss = wk2.tile([128, 1], F32, tag="ss")
                    nc.vector.reduce_sum(
                        ss[0:MPC, :], sq[0:MPC, :], axis=mybir.AxisListType.X
                    )
                    lt = wk2.tile([128, 1], F32, tag="lt")
                    nc.scalar.activation(
                        lt[0:MPC, :], ss[0:MPC, :], AF.Ln, bias=eps12_col[0:MPC, :]
                    )
                    rs = wk2.tile([128, 1], F32, tag="rs")
                    nc.scalar.activation(rs[0:MPC, :], lt[0:MPC, :], AF.Exp, scale=-0.5)
                    fin = wk2.tile([128, D], F32, tag="fin")
                    nc.vector.tensor_scalar(
                        out=fin[0:MPC, :],
                        in0=h2m[0:MPC, :],
                        scalar1=rs[0:MPC, :],
                        scalar2=None,
                        op0=ALU.mult,
                    )
                    nc.sync.dma_start(out=out_dram[k], in_=fin[0:MPC, :])

    _split_multiwaits(nc)
    return nc


# ----------------------------------------------------------------------------
# host side
# ----------------------------------------------------------------------------


def _prep_weights(inputs):
    f16 = np.float16
    f8 = mybir.dt.np(F8)

    def q8(w, on):
        if not on:
            return np.asarray(w, np.float32).astype(f16)
        return np.clip(np.asarray(w, np.float32) * WS, -224.0, 224.0).astype(f8)

    wq = np.stack(
        [inputs["W_qkv"][:, 0] / np.sqrt(DEPTH), inputs["W_qkv"][:, 3] / np.sqrt(DEPTH)],
        axis=1,
    )
    wk = np.stack([inputs["W_qkv"][:, 1], inputs["W_qkv"][:, 4]], axis=1)
    wv = np.concatenate([inputs["W_qkv"][:, 2], inputs["W_qkv"][:, 5]], axis=-1)
    return {
        "chain": np.zeros((C, MPC, U2), np.float32),
        "wembed": inputs["W_embed"].astype(f16),
        "wq": q8(wq, CONFIG["fp8_qkv"]),
        "wk": q8(wk, CONFIG["fp8_qkv"]),
        "wv": q8(wv, CONFIG["fp8_qkv"]),
        "wo": q8(inputs["W_o"], CONFIG["fp8_wo"]),
        "wf1": q8(inputs["W_ff1"], CONFIG["fp8_ffn"] in (True, "ffn1")),
        "wf2": q8(inputs["W_ff2"], CONFIG["fp8_ffn"] in (True, "ffn2")),
        "wh1": inputs["Wh1"].astype(f16),
        "wh2": inputs["Wh2"].astype(f16),
        "ident32": np.eye(128, dtype=np.float32),
        "ident16": np.eye(128, dtype=np.float16),
    }


def _check_trivial(inputs):
    z = [
        "b_embed",
        "b_qkv",
        "b_o",
        "b_ff1",
        "b_ff2",
        "bh1",
        "bh2",
        "ln1_b",
        "ln2_b",
    ]
    ok = all(np.abs(inputs[k]).max() == 0.0 for k in z)
    ok = ok and np.all(inputs["ln1_g"] == 1.0) and np.all(inputs["ln2_g"] == 1.0)
    if not ok:
        raise NotImplementedError(
            "kernel specialized for zero biases / unit layernorm gains (per spec)"
        )


def _mol_arrays(b_idx, inputs, S, perm=None):
    """Per-molecule prepped arrays, optionally token-permuted, truncated to S."""
    mol = np.asarray(inputs["mol_feat"][b_idx])
    adj = np.asarray(inputs["adj"][b_idx])
    dist = np.asarray(inputs["dist"][b_idx])
    mask = np.asarray(inputs["mask"][b_idx, 0, 0, :])
    if perm is not None:
        mol, adj, dist, mask = (
            mol[perm],
            adj[perm][:, perm],
            dist[perm][:, perm],
            mask[perm],
        )
    mol, adj, dist, mask = mol[:S], adj[:S, :S], dist[:S, :S], mask[:S]
    KC = len(_chunks(S))
    negc = np.full((KC, 128, 1), NEG, np.float32)
    poolm = np.zeros((KC, 128, 1), np.float16)
    negflat = (mask * NEG).astype(np.float32)
    poolflat = (mask == 0).astype(np.float32)
    for c, p0, pc in _chunks(S):
        negc[c, 0:pc, 0] = negflat[p0 : p0 + pc]
        poolm[c, 0:pc, 0] = poolflat[p0 : p0 + pc]
    # host-side softmax(dist + neg) over keys (layer-invariant)
    e = np.exp(dist.astype(np.float64)) * (mask == 0)[None, :]
    edn = e / e.sum(axis=1, keepdims=True)
    return {
        "mft": np.ascontiguousarray(mol.T).astype(np.float16),
        "adjT": np.ascontiguousarray(adj.T).astype(np.float16),
        "ednT": np.ascontiguousarray(edn.T).astype(np.float16),
        "negc": negc,
        "poolm": poolm,
    }


def plan(inputs):
    """Sort molecules by real-token count; slot s of every core gets one of the
    8 molecules ranked [8s, 8s+8); slot length = roundup8(max real in group)."""
    mask = np.asarray(inputs["mask"])[:, 0, 0, :]
    real = (mask == 0).sum(1)
    order = np.argsort(-real, kind="stable")
    slot_S, assign = [], [[0] * MPC for _ in range(NCORES)]
    for s in range(MPC):
        group = order[NCORES * s : NCORES * (s + 1)]
        Smax = int(min(((int(real[group].max()) + 7) // 8) * 8, S_FULL))
        slot_S.append(Smax)
        for c in range(NCORES):
            assign[c][s] = int(group[c])
    return slot_S, assign


def make_in_maps(inputs, slot_S, assign, compact=True):
    """assign[c][m] = molecule index for core c, slot m."""
    w = _prep_weights(inputs)
    mask_all = np.asarray(inputs["mask"])[:, 0, 0, :]
    in_maps = []
    for c in range(NCORES):
        im = dict(w)
        for m in range(MPC):
            b = assign[c][m]
            perm = None
            if compact:
                perm = np.argsort(mask_all[b], kind="stable")
            arrs = _mol_arrays(b, inputs, slot_S[m], perm=perm)
            for k, v in arrs.items():
                im[f"{k}{m}"] = v
        in_maps.append(im)
    return in_maps


def kernel(**inputs):
    _check_trivial(inputs)
    slot_S, assign = plan(inputs)
    nc = build_program(slot_S)
    in_maps = make_in_maps(inputs, slot_S, assign)
    from concourse.bass_utils import run_bass_kernel_spmd

    res = run_bass_kernel_spmd(nc, in_maps, core_ids=list(range(NCORES)))
    out = np.zeros((C, B, U2), np.float32)
    for c in range(NCORES):
        o = res.results[c]["out"]  # (C, MPC, U2)
        for m in range(MPC):
            out[:, assign[c][m], :] = o[:, m, :]
    return out


# revision 34
# speedup vs baseline: 1.0135x; 1.0135x over previous
"""Trainium2 Bass kernel for nn_EConly_85469849190489 (dual-branch molecular
transformer). Data-parallel over batch: 8 NeuronCores x 4 molecules each.

v2 layout strategy (per core):
  - softmax(dist+neg) precomputed on HOST (layer-invariant) -> ednT upload;
    the dist branch needs no device-side row-sum machinery
  - residual stream x kept tokens-major fp32 (LayerNorm via bn_stats);
    feature-major fp16 copies (xT) via PE transpose for matmul moving use
  - attention transposed: scores_T(k,q), pad mask as per-partition ACT bias,
    exp without max pass, A_raw^T feeds A@V with no transposes of A
  - softmax denominators via masked-ones matmuls packed into 32-row strips
    (per-strip start= resets, no PSUM memsets)
  - rsqrt via exp(-0.5*ln(x)) so the ACT engine never leaves the exp table
  - FFN2 computed tokens-major with hT as stationary (no ff2T transpose)
  - step-major emission skewed across molecules to keep engines busy
"""

import numpy as np
from contextlib import ExitStack

import concourse.bass as bass
import concourse.tile as tile
from concourse import mybir
from concourse.tile import TileContext, ScopedClock

F16 = mybir.dt.float16
F32 = mybir.dt.float32
F8 = mybir.dt.float8e4
DR = mybir.MatmulPerfMode.DoubleRow
AF = mybir.ActivationFunctionType
ALU = mybir.AluOpType

WS = 64.0  # host-side fp8 weight scale
QS = 16.0  # osc (attention output) fp8 scale

# scheduling/engine-placement knobs (tuned via TimelineSim)
CONFIG = {
    "ps_split": True,   # True: po/pr get their own PSUM pool (ps_o bufs=2)
    "ps_s_bufs": 2,     # score-tile double buffering (2 banks each)
    "ps_a_bufs": 2,     # general PSUM pool depth (1 bank each)
    "skew": (0, 2, 3),  # emission offsets of (proj, attn, ffn) stages
    "aw_pool": 0,       # every Nth Aw mul on gpsimd/Pool (0 = never)
    "ln_pool": False,   # LN applies on Pool
    "exp4": False,      # single [128,4,512] score tile + one 4-head exp
    "ps_o_bufs": 2,     # attention po/pr tile pairs in flight
    "fp8_qkv": False,   # xT + q/k/v weights in fp8 (DoubleRow projections)
    "fp8_wo": False,    # osc + W_o in fp8 (DoubleRow)
    "fp8_ffn": "ffn1",  # fp8 only in FFN1 (FFN2 in fp8 breaks accuracy)
}

B, S_FULL, F_IN, D, H, L, DFF, C = 32, 512, 64, 256, 8, 4, 1024, 4
U1, U2 = 512, 256
NH, DEPTH, DH = 4, 32, 128
NEG = -1.0e9
EPS_LN = 1.0e-6
NCORES = 8
MPC = 4  # molecules per core
P = 128

_PATCHED = False


def _patch_drain():
    """This walrus build allows only one sync-wait per CTRL instruction; the
    TileContext exit drain carries one wait per live semaphore. Spread the
    extras across single-wait SP nops."""
    global _PATCHED
    if _PATCHED:
        return

    def _drain_and_barrier(self, tick_clock, wait_clock):
        nc = self.nc
        drain_inst = nc.sync.drain()
        wait_clock.add_sem_waits(
            drain_inst.ins, ScopedClock({None: tick_clock.global_clock})
        )
        si = drain_inst.ins.sync_info
        waits = list(si.on_wait) if si is not None else []
        if len(waits) > 1:
            si.on_wait = waits[:1]
            drain_inst.ins.sync_info = si
            for w in waits[1:]:
                nop = nc.sync.nop(nofuse=True)
                nop.ins.sync_info = mybir.SyncInfo(on_wait=[w], on_update=[])
        nc.all_engine_barrier()
        popped = nc._tile_sem_poison_stack.pop()
        assert popped is self._sem_poison
        nc.clear_and_free_semaphores(list(self.sems.allocated().values()))
        nc.all_engine_barrier()

    TileContext._drain_and_barrier = _drain_and_barrier
    _PATCHED = True


def _split_multiwaits(nc):
    """This walrus build allows only one sync-wait per instruction: move extra
    waits onto same-engine nops placed immediately before the instruction."""
    n = 0
    for fn in nc.m.functions:
        for blk in fn.blocks:
            newl = []
            for inst in blk.instructions:
                si = inst.sync_info
                if si is not None and len(si.on_wait) > 1:
                    waits = list(si.on_wait)
                    for i, w in enumerate(waits[:-1]):
                        nop = mybir.InstNoOp(
                            name=f"{inst.name}-w{i}", ins=[], outs=[], engine=inst.engine
                        )
                        nop.sync_info = mybir.SyncInfo(on_wait=[w], on_update=[])
                        newl.append(nop)
                        n += 1
                    si.on_wait = waits[-1:]
                    inst.sync_info = si
                newl.append(inst)
            blk.instructions = newl
    return n


def _chunks(S):
    """[(c, p0, pc)] partition chunks covering S: offset p0, size pc<=128."""
    out = []
    c = 0
    while c * P < S:
        out.append((c, c * P, min(P, S - c * P)))
        c += 1
    return out


def build_program(slot_S, L_run=L, taps=(), time_loop=0):
    """Build the SPMD per-core program. slot_S: list of MPC sequence lengths
    (each a multiple of 8, <= 512). taps: debug tensor names to stream out."""
    _patch_drain()
    nc = bass.Bass(trn_type="TRN2", detect_race_conditions=False)

    dram = {}
    taps = set(taps)

    def tap(name, ap, shape=None):
        if name not in taps:
            return
        shape = list(shape if shape is not None else ap.shape)
        t = nc.dram_tensor("dbg_" + name, shape, ap.dtype, kind="ExternalOutput")
        nc.sync.dma_start(out=t[...], in_=ap)

    def din(name, shape, dt):
        dram[name] = nc.dram_tensor(name, list(shape), dt, kind="ExternalInput")
        return dram[name]

    for m, S in enumerate(slot_S):
        KC = len(_chunks(S))
        din(f"mft{m}", (64, S), F16)
        din(f"adjT{m}", (S, S), F16)
        din(f"ednT{m}", (S, S), F16)
        din(f"negc{m}", (KC, 128, 1), F32)
        din(f"poolm{m}", (KC, 128, 1), F16)
    din("wembed", (64, D), F16)
    QKV_DT = F8 if CONFIG["fp8_qkv"] else F16
    WO_DT = F8 if CONFIG["fp8_wo"] else F16
    ffn_mode = CONFIG["fp8_ffn"]
    FFN1_ON = ffn_mode in (True, "ffn1")
    FFN2_ON = ffn_mode in (True, "ffn2")
    FFN1_DT = F8 if FFN1_ON else F16
    FFN2_DT = F8 if FFN2_ON else F16
    din("wq", (L, 2, D, DH), QKV_DT)
    din("wk", (L, 2, D, DH), QKV_DT)
    din("wv", (L, D, 2 * DH), QKV_DT)
    din("wo", (L, D, D), WO_DT)
    din("wf1", (L, D, DFF), FFN1_DT)
    din("wf2", (L, DFF, D), FFN2_DT)
    din("wh1", (C, D, U1), F16)
    din("wh2", (C, U1, U2), F16)
    din("chain", (C, MPC, U2), F32)
    din("ident32", (128, 128), F32)
    din("ident16", (128, 128), F16)
    out_dram = nc.dram_tensor("out", [C, MPC, U2], F32, kind="ExternalOutput")

    MOLS = list(range(len(slot_S)))

    with TileContext(nc) as tc:
        with ExitStack() as ctx:
            pers = ctx.enter_context(tc.tile_pool(name="pers", bufs=1))
            wpool = ctx.enter_context(tc.tile_pool(name="wpool", bufs=2))
            wkq = ctx.enter_context(tc.tile_pool(name="wkq", bufs=1))
            wk2 = ctx.enter_context(tc.tile_pool(name="work", bufs=CONFIG.get("wk2_bufs", 2)))
            wk3 = ctx.enter_context(tc.tile_pool(name="work3", bufs=CONFIG.get("wk3_bufs", 6)))
            # PSUM budget is 8 banks total
            ps_s = ctx.enter_context(
                tc.tile_pool(name="ps_s", bufs=CONFIG["ps_s_bufs"], space="PSUM")
            )
            if CONFIG["ps_split"]:
                ps_o = ctx.enter_context(
                    tc.tile_pool(name="ps_o", bufs=CONFIG["ps_o_bufs"], space="PSUM")
                )
            ps_a = ctx.enter_context(
                tc.tile_pool(name="ps_a", bufs=CONFIG["ps_a_bufs"], space="PSUM")
            )
            if not CONFIG["ps_split"]:
                ps_o = ps_a

            # ---- persistent constants / state ----
            ident_32 = pers.tile([128, 128], F32, tag="id32")
            nc.sync.dma_start(out=ident_32, in_=dram["ident32"][:, :])
            ident_16 = pers.tile([128, 128], F16, tag="id16")
            nc.sync.dma_start(out=ident_16, in_=dram["ident16"][:, :])
            ones16 = pers.tile([128, 32], F16, tag="ones16")
            nc.vector.memset(ones16, 1.0)
            eps_col = pers.tile([128, 1], F32, tag="eps")
            nc.vector.memset(eps_col, EPS_LN)
            eps12_col = pers.tile([128, 1], F32, tag="eps12")
            nc.vector.memset(eps12_col, 1.0e-12)
            chain_sb = pers.tile([C, MPC * U2], F32, tag="chain")
            nc.sync.dma_start(
                out=chain_sb, in_=dram["chain"].rearrange("k m u -> k (m u)")
            )

            negc, poolm, xT, xtok, adjT, ednT, mft = [], [], [], [], [], [], []
            for m, S in enumerate(slot_S):
                ch = _chunks(S)
                KC = len(ch)
                t = pers.tile([128, KC, 1], F32, tag=f"negc{m}", name=f"negc{m}")
                nc.sync.dma_start(
                    out=t, in_=dram[f"negc{m}"].rearrange("c p one -> p c one")
                )
                negc.append(t)
                t = pers.tile([128, KC, 1], F16, tag=f"poolm{m}", name=f"poolm{m}")
                nc.sync.dma_start(
                    out=t, in_=dram[f"poolm{m}"].rearrange("c p one -> p c one")
                )
                poolm.append(t)
                xT.append(pers.tile([128, 2, 512], QKV_DT, tag=f"xT{m}", name=f"xT{m}"))
                xtok.append(pers.tile([128, KC, D], F16, tag=f"xtok{m}", name=f"xtok{m}"))
                t = pers.tile([128, KC, S], F16, tag=f"adjT{m}", name=f"adjT{m}")
                for c, p0, pc in ch:
                    nc.sync.dma_start(
                        out=t[0:pc, c, :], in_=dram[f"adjT{m}"][p0 : p0 + pc, :]
                    )
                adjT.append(t)
                t = pers.tile([128, KC, S], F16, tag=f"ednT{m}", name=f"ednT{m}")
                for c, p0, pc in ch:
                    nc.sync.dma_start(
                        out=t[0:pc, c, :], in_=dram[f"ednT{m}"][p0 : p0 + pc, :]
                    )
                ednT.append(t)
                t = pers.tile([128, S], F16, tag=f"mft{m}", name=f"mft{m}")
                nc.sync.dma_start(out=t[0:64, :], in_=dram[f"mft{m}"][:, :])
                mft.append(t)

            wh1_sb = pers.tile([128, C, 2, U1], F16, tag="wh1")
            nc.sync.dma_start(
                out=wh1_sb, in_=dram["wh1"].rearrange("k (c p) n -> p k c n", p=128)
            )
            wh2_sb = pers.tile([128, C, 4, U2], F16, tag="wh2")
            nc.sync.dma_start(
                out=wh2_sb, in_=dram["wh2"].rearrange("k (c p) n -> p k c n", p=128)
            )
            wembed_sb = pers.tile([128, D], F16, tag="wembed")
            nc.sync.dma_start(out=wembed_sb[0:64, :], in_=dram["wembed"][:, :])

            prow = pers.tile([128, MPC * D], F32, tag="prow")
            MM = nc.tensor.matmul

            # per-mol tiles that live across layer steps
            qkT = [
                wkq.tile([128, 4, S], F16, tag=f"qkT{m}", name=f"qkT{m}")
                for m, S in enumerate(slot_S)
            ]
            Vt = [
                wkq.tile([128, KC_, 2 * DH], F16, tag=f"Vt{m}", name=f"Vt{m}")
                for m, (S, KC_) in enumerate(
                    (S, len(_chunks(S))) for S in slot_S
                )
            ]
            osc = [
                wkq.tile([128, 2, 512], WO_DT, tag=f"osc{m}", name=f"osc{m}")
                for m, S in enumerate(slot_S)
            ]

            from contextlib import nullcontext

            with (tc.For_i(0, time_loop, 1) if time_loop else nullcontext()):
                # ---- phase 0: embed ----
                for m, S in enumerate(slot_S):
                    ch = _chunks(S)
                    for c2 in range(2):
                        pse = ps_a.tile([128, 512], F32, tag="a")
                        MM(
                            pse[:, 0:S],
                            wembed_sb[0:64, c2 * 128 : (c2 + 1) * 128],
                            mft[m][0:64, :],
                            start=True,
                            stop=True,
                        )
                        nc.scalar.copy(out=xT[m][:, c2, 0:S], in_=pse[:, 0:S])
                    for c, p0, pc in ch:
                        pse = ps_a.tile([128, 512], F32, tag="a")
                        MM(
                            pse[0:pc, 0:D],
                            mft[m][0:64, p0 : p0 + pc],
                            wembed_sb[0:64, :],
                            start=True,
                            stop=True,
                        )
                        nc.vector.tensor_copy(out=xtok[m][0:pc, c, :], in_=pse[0:pc, 0:D])

                tap("xT0", xT[0][:, :, :])
                tap("xtok0", xtok[0][:, :, :])

                # ---- layers ----
                for l in range(L_run):
                    wq_sb = wpool.tile([128, 2, 2, DH], QKV_DT, tag="wq")
                    nc.sync.dma_start(
                        out=wq_sb,
                        in_=dram["wq"][l].rearrange("b (c p) n -> p b c n", p=128),
                    )
                    wk_sb = wpool.tile([128, 2, 2, DH], QKV_DT, tag="wk")
                    nc.sync.dma_start(
                        out=wk_sb,
                        in_=dram["wk"][l].rearrange("b (c p) n -> p b c n", p=128),
                    )
                    wv_sb = wpool.tile([128, 2, 2 * DH], QKV_DT, tag="wv")
                    nc.sync.dma_start(
                        out=wv_sb, in_=dram["wv"][l].rearrange("(c p) n -> p c n", p=128)
                    )
                    wo_sb = wpool.tile([128, 2, D], WO_DT, tag="wo")
                    nc.sync.dma_start(
                        out=wo_sb, in_=dram["wo"][l].rearrange("(c p) n -> p c n", p=128)
                    )
                    wf1_sb = wpool.tile([128, 2, DFF], FFN1_DT, tag="wf1")
                    nc.sync.dma_start(
                        out=wf1_sb, in_=dram["wf1"][l].rearrange("(c p) n -> p c n", p=128)
                    )
                    wf2_sb = wpool.tile([128, 8, D], FFN2_DT, tag="wf2")
                    nc.sync.dma_start(
                        out=wf2_sb, in_=dram["wf2"][l].rearrange("(c p) n -> p c n", p=128)
                    )

                    # ---- S1+S2: q/k/v projections for mol m ----
                    def proj_qkv(m):
                        S = slot_S[m]
                        ch = _chunks(S)
                        for br in range(2):
                            for which, w_sb in ((0, wq_sb), (1, wk_sb)):
                                pp = ps_a.tile([128, 512], F32, tag="a")
                                if CONFIG["fp8_qkv"]:
                                    MM(
                                        pp[:, 0:S],
                                        w_sb[:, br, :, :],
                                        xT[m][:, :, 0:S],
                                        start=True,
                                        stop=True,
                                        perf_mode=DR,
                                    )
                                else:
                                    for cc in range(2):
                                        MM(
                                            pp[:, 0:S],
                                            w_sb[:, br, cc, :],
                                            xT[m][:, cc, 0:S],
                                            start=(cc == 0),
                                            stop=(cc == 1),
                                        )
                                # qkT slots: 0,1 = qT br0/br1; 2,3 = kT br0/br1
                                slot = which * 2 + br
                                qs = (1.0 / WS) if CONFIG["fp8_qkv"] else 1.0
                                if which == 0:
                                    nc.scalar.activation(
                                        qkT[m][:, slot, :],
                                        pp[:, 0:S],
                                        AF.Copy,
                                        scale=qs,
                                    )
                                else:
                                    nc.vector.tensor_scalar(
                                        out=qkT[m][:, slot, :],
                                        in0=pp[:, 0:S],
                                        scalar1=qs,
                                        scalar2=None,
                                        op0=ALU.mult,
                                    )
                        for c, p0, pc in ch:
                            pv = ps_a.tile([128, 512], F32, tag="a")
                            if CONFIG["fp8_qkv"]:
                                MM(
                                    pv[0:pc, 0 : 2 * DH],
                                    xT[m][:, :, p0 : p0 + pc],
                                    wv_sb[:, :, :],
                                    start=True,
                                    stop=True,
                                    perf_mode=DR,
                                )
                            else:
                                for cc in range(2):
                                    MM(
                                        pv[0:pc, 0 : 2 * DH],
                                        xT[m][:, cc, p0 : p0 + pc],
                                        wv_sb[:, cc, :],
                                        start=(cc == 0),
                                        stop=(cc == 1),
                                    )
                            nc.vector.tensor_scalar(
                                out=Vt[m][0:pc, c, :],
                                in0=pv[0:pc, 0 : 2 * DH],
                                scalar1=(1.0 / WS) if CONFIG["fp8_qkv"] else 1.0,
                                scalar2=None,
                                op0=ALU.mult,
                            )

                    # ---- S3: attention both branches for mol m ----
                    def attn(m):
                        S = slot_S[m]
                        ch = _chunks(S)
                        KC = len(ch)
                        for br in range(2):
                            qT = qkT[m][:, 0 + br, :]
                            kT = qkT[m][:, 2 + br, :]
                            MT = ednT[m] if br == 0 else adjT[m]
                            otag = "o" if CONFIG["ps_split"] else "a"
                            # phase 1: scores -> exp -> masked product, per chunk
                            PTs, Aws = [], []
                            for c, p0, pc in ch:
                                PT = wk3.tile([128, 4, S], F16, tag="PT")
                                for pair in range(2):
                                    ps = ps_s.tile([128, 2, 512], F32, tag="s")
                                    for i in range(2):
                                        h = 2 * pair + i
                                        MM(
                                            ps[0:pc, i, 0:S],
                                            kT[32 * h : 32 * h + 32, p0 : p0 + pc],
                                            qT[32 * h : 32 * h + 32, :],
                                            start=True,
                                            stop=True,
                                            tile_position=(32 * h, 0),
                                        )
                                    nc.scalar.activation(
                                        PT[0:pc, 2 * pair : 2 * pair + 2, :],
                                        ps[0:pc, :, 0:S],
                                        AF.Exp,
                                        bias=negc[m][0:pc, c, :],
                                        scale=1.0,
                                    )
                                Aw = wk3.tile([128, 4, S], F16, tag="Aw")
                                nc.vector.tensor_mul(
                                    Aw[0:pc, :, :],
                                    PT[0:pc, :, :],
                                    MT[0:pc, c : c + 1, :].broadcast_to((pc, 4, S)),
                                )
                                PTs.append(PT)
                                Aws.append(Aw)
                            # phase 2: denominator + A@V bursts (short PSUM residency)
                            po = ps_o.tile([128, 512], F32, tag=otag)
                            pr = ps_o.tile([128, 512], F32, tag=otag)
                            for c, p0, pc in ch:
                                for h in range(4):
                                    MM(
                                        pr[32 * h : 32 * h + 32, 0:S],
                                        ones16[0:pc, 0:32],
                                        PTs[c][0:pc, h, :],
                                        start=(c == 0),
                                        stop=(c == KC - 1),
                                        tile_position=(0, 32 * h),
                                        skip_group_check=True,
                                    )
                                    MM(
                                        po[32 * h : 32 * h + 32, 0:S],
                                        Vt[m][
                                            0:pc,
                                            c,
                                            br * DH + 32 * h : br * DH + 32 * h + 32,
                                        ],
                                        Aws[c][0:pc, h, :],
                                        start=(c == 0),
                                        stop=(c == KC - 1),
                                        tile_position=(0, 32 * h),
                                        skip_group_check=True,
                                    )
                            recipb = wk2.tile([128, S], F32, tag="recipb")
                            nc.vector.reciprocal(out=recipb[:, :], in_=pr[:, 0:S])
                            nc.vector.scalar_tensor_tensor(
                                out=osc[m][:, br, 0:S],
                                in0=po[:, 0:S],
                                scalar=QS if CONFIG["fp8_wo"] else 1.0,
                                in1=recipb[:, :],
                                op0=ALU.mult,
                                op1=ALU.mult,
                            )
                        if l == 0 and m == 0:
                            tap("osc0", osc[m][:, :, :])

                    # ---- S4+S5+S6: Wo+LN1, FFN1, FFN2+LN2, next xT for mol m ----
                    def ffn_block(m):
                        S = slot_S[m]
                        ch = _chunks(S)
                        KC = len(ch)
                        # Wo + residual + LN1
                        xres = wk2.tile([128, KC, D], F16, tag="xres")
                        xln1 = wk3.tile([128, KC, D], F16, tag="xln1")
                        mv = wk2.tile([128, KC, 2], F32, tag="mv")
                        if S % 128:
                            pal = ((S % 128) // 32) * 32
                            nc.vector.memset(xres[pal:128, KC - 1, :], 0.0)
                        for c, p0, pc in ch:
                            pw = ps_a.tile([128, 512], F32, tag="a")
                            if CONFIG["fp8_wo"]:
                                MM(
                                    pw[0:pc, 0:D],
                                    osc[m][:, :, p0 : p0 + pc],
                                    wo_sb[:, :, :],
                                    start=True,
                                    stop=True,
                                    perf_mode=DR,
                                )
                            else:
                                for cc in range(2):
                                    MM(
                                        pw[0:pc, 0:D],
                                        osc[m][:, cc, p0 : p0 + pc],
                                        wo_sb[:, cc, :],
                                        start=(cc == 0),
                                        stop=(cc == 1),
                                    )
                            nc.vector.scalar_tensor_tensor(
                                out=xres[0:pc, c, :],
                                in0=pw[0:pc, 0:D],
                                scalar=(1.0 / (WS * QS)) if CONFIG["fp8_wo"] else 1.0,
                                in1=xtok[m][0:pc, c, :],
                                op0=ALU.mult,
                                op1=ALU.add,
                            )
                        bst = wk3.tile([128, KC, 6], F32, tag="bst")
                        for c in range(KC):
                            nc.vector.bn_stats(out=bst[:, c, :], in_=xres[:, c, :])
                            nc.vector.bn_aggr(out=mv[:, c, :], in_=bst[:, c, :])
                        if l == 0 and m == 0:
                            tap("xres0", xres[:, :, :])
                        # rstd = exp(-0.5*ln(var+eps)) - stays in the exp table
                        sd = wk2.tile([128, KC], F32, tag="sd")
                        nc.scalar.activation(
                            sd[:, :], mv[:, :, 1], AF.Ln, bias=eps_col[:, :], scale=1.0
                        )
                        rstd = wk2.tile([128, KC], F32, tag="rstd")
                        nc.scalar.activation(rstd[:, :], sd[:, :], AF.Exp, scale=-0.5)
                        for c, p0, pc in ch:
                            nc.vector.tensor_scalar(
                                out=xln1[0:pc, c, :],
                                in0=xres[0:pc, c, :],
                                scalar1=mv[0:pc, c, 0:1],
                                scalar2=rstd[0:pc, c : c + 1],
                                op0=ALU.subtract,
                                op1=ALU.mult,
                            )
                        if l == 0 and m == 0:
                            tap("xln1_0", xln1[:, :, :])
                        # x_ln1^T (feature-major fp8) via PE transpose (f16 in)
                        xln1T = wk2.tile([128, 2, 512], FFN1_DT, tag="xln1T")
                        for cc in range(2):
                            pt = ps_a.tile([128, 512], F16, tag="a")
                            for c, p0, pc in ch:
                                nc.tensor.transpose(
                                    pt[:, p0 : p0 + pc],
                                    xln1[0:pc, c, cc * 128 : (cc + 1) * 128],
                                    ident_16[0:pc, 0:pc],
                                )
                            nc.scalar.copy(out=xln1T[:, cc, 0:S], in_=pt[:, 0:S])
                        if l == 0 and m == 0:
                            tap("xln1T0", xln1T[:, :, :])
                        # FFN1 (feature-major) with relu -> h^T fp8
                        hT = wk2.tile([128, 8, 512], FFN2_DT, tag="hT")
                        for hc in range(8):
                            pf = ps_a.tile([128, 512], F32, tag="a")
                            if FFN1_ON:
                                MM(
                                    pf[:, 0:S],
                                    wf1_sb[:, :, hc * 128 : (hc + 1) * 128],
                                    xln1T[:, :, 0:S],
                                    start=True,
                                    stop=True,
                                    perf_mode=DR,
                                )
                            else:
                                for cc in range(2):
                                    MM(
                                        pf[:, 0:S],
                                        wf1_sb[:, cc, hc * 128 : (hc + 1) * 128],
                                        xln1T[:, cc, 0:S],
                                        start=(cc == 0),
                                        stop=(cc == 1),
                                    )
                            hs = (1.0 / WS) if FFN1_ON else 1.0
                            if hc % 2 == 0:
                                nc.scalar.activation(
                                    hT[:, hc, 0:S], pf[:, 0:S], AF.Relu, scale=hs
                                )
                            else:
                                nc.vector.tensor_scalar(
                                    out=hT[:, hc, 0:S],
                                    in0=pf[:, 0:S],
                                    scalar1=0.0,
                                    scalar2=hs,
                                    op0=ALU.max,
                                    op1=ALU.mult,
                                )
                        if l == 0 and m == 0:
                            tap("hT0", hT[:, :, :])
                        # FFN2 tokens-major: stationary = hT chunk, moving = wf2
                        mv2 = wk2.tile([128, KC, 2], F32, tag="mv2")
                        xres2 = wk2.tile([128, KC, D], F16, tag="xres2")
                        if S % 128:
                            pal = ((S % 128) // 32) * 32
                            nc.vector.memset(xres2[pal:128, KC - 1, :], 0.0)
                        for c, p0, pc in ch:
                            pf = ps_a.tile([128, 512], F32, tag="a")
                            if FFN2_ON:
                                for j in range(4):
                                    MM(
                                        pf[0:pc, 0:D],
                                        hT[:, 2 * j : 2 * j + 2, p0 : p0 + pc],
                                        wf2_sb[:, 2 * j : 2 * j + 2, :],
                                        start=(j == 0),
                                        stop=(j == 3),
                                        perf_mode=DR,
                                    )
                            else:
                                for hc in range(8):
                                    MM(
                                        pf[0:pc, 0:D],
                                        hT[:, hc, p0 : p0 + pc],
                                        wf2_sb[:, hc, :],
                                        start=(hc == 0),
                                        stop=(hc == 7),
                                    )
                            nc.vector.scalar_tensor_tensor(
                                out=xres2[0:pc, c, :],
                                in0=pf[0:pc, 0:D],
                                scalar=(1.0 / WS) if FFN2_ON else 1.0,
                                in1=xln1[0:pc, c, :],
                                op0=ALU.mult,
                                op1=ALU.add,
                            )
                        bst = wk3.tile([128, KC, 6], F32, tag="bst")
                        for c in range(KC):
                            nc.vector.bn_stats(out=bst[:, c, :], in_=xres2[:, c, :])
                            nc.vector.bn_aggr(out=mv2[:, c, :], in_=bst[:, c, :])
                        sd2 = wk2.tile([128, KC], F32, tag="sd2")
                        nc.scalar.activation(
                            sd2[:, :], mv2[:, :, 1], AF.Ln, bias=eps_col[:, :], scale=1.0
                        )
                        rstd2 = wk2.tile([128, KC], F32, tag="rstd2")
                        nc.scalar.activation(rstd2[:, :], sd2[:, :], AF.Exp, scale=-0.5)
                        for c, p0, pc in ch:
                            nc.vector.tensor_scalar(
                                out=xtok[m][0:pc, c, :],
                                in0=xres2[0:pc, c, :],
                                scalar1=mv2[0:pc, c, 0:1],
                                scalar2=rstd2[0:pc, c : c + 1],
                                op0=ALU.subtract,
                                op1=ALU.mult,
                            )
                        if l == 0 and m == 0:
                            tap("xtok_l0", xtok[m][:, :, :])
                        if l < L_run - 1:
                            for cc in range(2):
                                pt = ps_a.tile([128, 512], F16, tag="a")
                                for c, p0, pc in ch:
                                    nc.tensor.transpose(
                                        pt[:, p0 : p0 + pc],
                                        xtok[m][0:pc, c, cc * 128 : (cc + 1) * 128],
                                        ident_16[0:pc, 0:pc],
                                    )
                                nc.scalar.copy(out=xT[m][:, cc, 0:S], in_=pt[:, 0:S])

                    def pool_mol(m):
                        S = slot_S[m]
                        ch = _chunks(S)
                        pp = ps_a.tile([128, 512], F32, tag="a")
                        for c, p0, pc in ch:
                            MM(
                                pp[0:1, 0:D],
                                poolm[m][0:pc, c, :],
                                xtok[m][0:pc, c, :],
                                start=(c == 0),
                                stop=(c == len(ch) - 1),
                            )
                        nc.vector.tensor_copy(
                            out=prow[0:1, m * D : (m + 1) * D], in_=pp[0:1, 0:D]
                        )

                    # skewed emission: proj runs ahead of attn, attn ahead of
                    # the ffn block, so each engine sees independent work from
                    # neighbouring molecules between dependent stages
                    stages = [proj_qkv, attn, ffn_block]
                    offs = list(CONFIG["skew"])
                    if l == L_run - 1:
                        stages.append(pool_mol)
                        offs.append(offs[-1] + 1)
                    for t in range(len(MOLS) + max(offs)):
                        for si in range(len(stages)):
                            mi = t - offs[si]
                            if 0 <= mi < len(MOLS):
                                stages[si](mi)

                    if l == 0:
                        tap("qkT0", qkT[0][:, :, :])
                        tap("Vt0", Vt[0][:, :, :])

                # ---- contrastive heads (pooling ran as a last-layer stage) ----
                ones32c = pers.tile([128, 1], F32, tag="ones32c")
                nc.vector.memset(ones32c, 1.0)
                # pooled^T (256 x MPC) fp16 via K=1 matmuls off the partition-0 row
                pT_sb = pers.tile([128, 2, MPC], F16, tag="pT")
                ptp = ps_a.tile([128, 512], F32, tag="a")
                for cc in range(2):
                    for m in range(MPC):
                        MM(
                            ptp[:, cc * MPC + m : cc * MPC + m + 1],
                            prow[0:1, m * D + cc * 128 : m * D + (cc + 1) * 128],
                            ones32c[0:1, 0:1],
                            start=(cc == 0 and m == 0),
                            stop=(cc == 1 and m == MPC - 1),
                            skip_group_check=True,
                        )
                nc.vector.tensor_copy(
                    out=pT_sb[:, :, :],
                    in_=ptp[:, 0 : 2 * MPC].rearrange("p (c m) -> p c m", c=2),
                )
                for k in range(C):
                    h1 = wk2.tile([128, 4, MPC], F16, tag="h1")
                    for u in range(4):
                        ph = ps_a.tile([128, 512], F32, tag="a")
                        for cc in range(2):
                            MM(
                                ph[:, 0:MPC],
                                wh1_sb[:, k, cc, u * 128 : (u + 1) * 128],
                                pT_sb[:, cc, :],
                                start=(cc == 0),
                                stop=(cc == 1),
                            )
                        nc.scalar.activation(h1[:, u, :], ph[:, 0:MPC], AF.Relu)
                    h2m = wk2.tile([128, D], F32, tag="h2m")
                    ph2 = ps_a.tile([128, 512], F32, tag="a")
                    for oc in range(2):
                        for uc in range(4):
                            MM(
                                ph2[:, oc * MPC : (oc + 1) * MPC],
                                wh2_sb[:, k, uc, oc * 128 : (oc + 1) * 128],
                                h1[:, uc, :],
                                start=(uc == 0),
                                stop=(uc == 3),
                            )
                    # relu into fp16 tile, transpose to (MPC x 256) rows
                    h2f = wk2.tile([128, 2, MPC], F16, tag="h2f")
                    for oc in range(2):
                        nc.scalar.activation(
                            h2f[:, oc, :], ph2[:, oc * MPC : (oc + 1) * MPC], AF.Relu
                        )
                    pht = ps_a.tile([128, 512], F16, tag="a")
                    for oc in range(2):
                        nc.tensor.transpose(
                            pht[0:MPC, oc * 128 : (oc + 1) * 128],
                            h2f[:, oc, :],
                            ident_16[:, :],
                        )
                    nc.vector.tensor_copy(out=h2m[0:MPC, :], in_=pht[0:MPC, 0:D])
                    # l2 normalize rows: rs = exp(-0.5*ln(ss+1e-12))
                    sq = wk2.tile([128, D], F32, tag="sq")
                    nc.vector.tensor_mul(sq[0:MPC, :], h2m[0:MPC, :], h2m[0:MPC, :])
                    ss = wk2.tile([128, 1], F32, tag="ss")
                    nc.vector.reduce_sum(
                        ss[0:MPC, :], sq[0:MPC, :], axis=mybir.AxisListType.X
                    )
                    lt = wk2.tile([128, 1], F32, tag="lt")
                    nc.scalar.activation(
                        lt[0:MPC, :], ss[0:MPC, :], AF.Ln, bias=eps12_col[0:MPC, :]
                    )
                    rs = wk2.tile([128, 1], F32, tag="rs")
                    nc.scalar.activation(rs[0:MPC, :], lt[0:MPC, :], AF.Exp, scale=-0.5)
                    fin = wk2.tile([128, D], F32, tag="fin")
                    nc.vector.tensor_scalar(
                        out=fin[0:MPC, :],
                        in0=h2m[0:MPC, :],
                        scalar1=rs[0:MPC, :],
                        scalar2=None,
                        op0=ALU.mult,
                    )
                    nc.sync.dma_start(out=out_dram[k], in_=fin[0:MPC, :])

    _split_multiwaits(nc)
    return nc


# ----------------------------------------------------------------------------
# host side
# ----------------------------------------------------------------------------


def _prep_weights(inputs):
    f16 = np.float16
    f8 = mybir.dt.np(F8)

    def q8(w, on):
        if not on:
            return np.asarray(w, np.float32).astype(f16)
        return np.clip(np.asarray(w, np.float32) * WS, -224.0, 224.0).astype(f8)

    wq = np.stack(
        [inputs["W_qkv"][:, 0] / np.sqrt(DEPTH), inputs["W_qkv"][:, 3] / np.sqrt(DEPTH)],
        axis=1,
    )
    wk = np.stack([inputs["W_qkv"][:, 1], inputs["W_qkv"][:, 4]], axis=1)
    wv = np.concatenate([inputs["W_qkv"][:, 2], inputs["W_qkv"][:, 5]], axis=-1)
    return {
        "chain": np.zeros((C, MPC, U2), np.float32),
        "wembed": inputs["W_embed"].astype(f16),
        "wq": q8(wq, CONFIG["fp8_qkv"]),
        "wk": q8(wk, CONFIG["fp8_qkv"]),
        "wv": q8(wv, CONFIG["fp8_qkv"]),
        "wo": q8(inputs["W_o"], CONFIG["fp8_wo"]),
        "wf1": q8(inputs["W_ff1"], CONFIG["fp8_ffn"] in (True, "ffn1")),
        "wf2": q8(inputs["W_ff2"], CONFIG["fp8_ffn"] in (True, "ffn2")),
        "wh1": inputs["Wh1"].astype(f16),
        "wh2": inputs["Wh2"].astype(f16),
        "ident32": np.eye(128, dtype=np.float32),
        "ident16": np.eye(128, dtype=np.float16),
    }


def _check_trivial(inputs):
    z = [
        "b_embed",
        "b_qkv",
        "b_o",
        "b_ff1",
        "b_ff2",
        "bh1",
        "bh2",
        "ln1_b",
        "ln2_b",
    ]
    ok = all(np.abs(inputs[k]).max() == 0.0 for k in z)
    ok = ok and np.all(inputs["ln1_g"] == 1.0) and np.all(inputs["ln2_g"] == 1.0)
    if not ok:
        raise NotImplementedError(
            "kernel specialized for zero biases / unit layernorm gains (per spec)"
        )


def _mol_arrays(b_idx, inputs, S, perm=None):
    """Per-molecule prepped arrays, optionally token-permuted, truncated to S."""
    mol = np.asarray(inputs["mol_feat"][b_idx])
    adj = np.asarray(inputs["adj"][b_idx])
    dist = np.asarray(inputs["dist"][b_idx])
    mask = np.asarray(inputs["mask"][b_idx, 0, 0, :])
    if perm is not None:
        mol, adj, dist, mask = (
            mol[perm],
            adj[perm][:, perm],
            dist[perm][:, perm],
            mask[perm],
        )
    mol, adj, dist, mask = mol[:S], adj[:S, :S], dist[:S, :S], mask[:S]
    KC = len(_chunks(S))
    negc = np.full((KC, 128, 1), NEG, np.float32)
    poolm = np.zeros((KC, 128, 1), np.float16)
    negflat = (mask * NEG).astype(np.float32)
    poolflat = (mask == 0).astype(np.float32)
    for c, p0, pc in _chunks(S):
        negc[c, 0:pc, 0] = negflat[p0 : p0 + pc]
        poolm[c, 0:pc, 0] = poolflat[p0 : p0 + pc]
    # host-side softmax(dist + neg) over keys (layer-invariant)
    e = np.exp(dist.astype(np.float64)) * (mask == 0)[None, :]
    edn = e / e.sum(axis=1, keepdims=True)
    return {
        "mft": np.ascontiguousarray(mol.T).astype(np.float16),
        "adjT": np.ascontiguousarray(adj.T).astype(np.float16),
        "ednT": np.ascontiguousarray(edn.T).astype(np.float16),
        "negc": negc,
        "poolm": poolm,
    }


def plan(inputs):
    """Sort molecules by real-token count; slot s of every core gets one of the
    8 molecules ranked [8s, 8s+8); slot length = roundup8(max real in group)."""
    mask = np.asarray(inputs["mask"])[:, 0, 0, :]
    real = (mask == 0).sum(1)
    order = np.argsort(-real, kind="stable")
    slot_S, assign = [], [[0] * MPC for _ in range(NCORES)]
    for s in range(MPC):
        group = order[NCORES * s : NCORES * (s + 1)]
        Smax = int(min(((int(real[group].max()) + 7) // 8) * 8, S_FULL))
        slot_S.append(Smax)
        for c in range(NCORES):
            assign[c][s] = int(group[c])
    return slot_S, assign


def make_in_maps(inputs, slot_S, assign, compact=True):
    """assign[c][m] = molecule index for core c, slot m."""
    w = _prep_weights(inputs)
    mask_all = np.asarray(inputs["mask"])[:, 0, 0, :]
    in_maps = []
    for c in range(NCORES):
        im = dict(w)
        for m in range(MPC):
            b = assign[c][m]
            perm = None
            if compact:
                perm = np.argsort(mask_all[b], kind="stable")
            arrs = _mol_arrays(b, inputs, slot_S[m], perm=perm)
            for k, v in arrs.items():
                im[f"{k}{m}"] = v
        in_maps.append(im)
    return in_maps


def kernel(**inputs):
    _check_trivial(inputs)
    slot_S, assign = plan(inputs)
    nc = build_program(slot_S)
    in_maps = make_in_maps(inputs, slot_S, assign)
    from concourse.bass_utils import run_bass_kernel_spmd

    res = run_bass_kernel_spmd(nc, in_maps, core_ids=list(range(NCORES)))
    out = np.zeros((C, B, U2), np.float32)
    for c in range(NCORES):
        o = res.results[c]["out"]  # (C, MPC, U2)
        for m in range(MPC):
            out[:, assign[c][m], :] = o[:, m, :]
    return out


# revision 38
# speedup vs baseline: 1.0137x; 1.0002x over previous
"""Trainium2 Bass kernel for nn_EConly_85469849190489 (dual-branch molecular
transformer). Data-parallel over batch: 8 NeuronCores x 4 molecules each.

v2 layout strategy (per core):
  - softmax(dist+neg) precomputed on HOST (layer-invariant) -> ednT upload;
    the dist branch needs no device-side row-sum machinery
  - residual stream x kept tokens-major fp32 (LayerNorm via bn_stats);
    feature-major fp16 copies (xT) via PE transpose for matmul moving use
  - attention transposed: scores_T(k,q), pad mask as per-partition ACT bias,
    exp without max pass, A_raw^T feeds A@V with no transposes of A
  - softmax denominators via masked-ones matmuls packed into 32-row strips
    (per-strip start= resets, no PSUM memsets)
  - rsqrt via exp(-0.5*ln(x)) so the ACT engine never leaves the exp table
  - FFN2 computed tokens-major with hT as stationary (no ff2T transpose)
  - step-major emission skewed across molecules to keep engines busy
"""

import numpy as np
from contextlib import ExitStack

import concourse.bass as bass
import concourse.tile as tile
from concourse import mybir
from concourse.tile import TileContext, ScopedClock

F16 = mybir.dt.float16
F32 = mybir.dt.float32
F8 = mybir.dt.float8e4
DR = mybir.MatmulPerfMode.DoubleRow
AF = mybir.ActivationFunctionType
ALU = mybir.AluOpType

WS = 64.0  # host-side fp8 weight scale
QS = 16.0  # osc (attention output) fp8 scale

# scheduling/engine-placement knobs (tuned via TimelineSim)
CONFIG = {
    "ps_split": True,   # True: po/pr get their own PSUM pool (ps_o bufs=2)
    "ps_s_bufs": 2,     # score-tile double buffering (2 banks each)
    "ps_a_bufs": 2,     # general PSUM pool depth (1 bank each)
    "skew": (0, 2, 3),  # emission offsets of (proj, attn, ffn) stages
    "aw_pool": 0,       # every Nth Aw mul on gpsimd/Pool (0 = never)
    "ln_pool": False,   # LN applies on Pool
    "exp4": False,      # single [128,4,512] score tile + one 4-head exp
    "ps_o_bufs": 2,     # attention po/pr tile pairs in flight
    "br_inter": False,  # interleave the two branches' score/exp/mul chunks
    "fp8_qkv": False,   # xT + q/k/v weights in fp8 (DoubleRow projections)
    "fp8_wo": False,    # osc + W_o in fp8 (DoubleRow)
    "fp8_ffn": "ffn1",  # fp8 only in FFN1 (FFN2 in fp8 breaks accuracy)
}

B, S_FULL, F_IN, D, H, L, DFF, C = 32, 512, 64, 256, 8, 4, 1024, 4
U1, U2 = 512, 256
NH, DEPTH, DH = 4, 32, 128
NEG = -1.0e9
EPS_LN = 1.0e-6
NCORES = 8
MPC = 4  # molecules per core
P = 128

_PATCHED = False


def _patch_drain():
    """This walrus build allows only one sync-wait per CTRL instruction; the
    TileContext exit drain carries one wait per live semaphore. Spread the
    extras across single-wait SP nops."""
    global _PATCHED
    if _PATCHED:
        return

    def _drain_and_barrier(self, tick_clock, wait_clock):
        nc = self.nc
        drain_inst = nc.sync.drain()
        wait_clock.add_sem_waits(
            drain_inst.ins, ScopedClock({None: tick_clock.global_clock})
        )
        si = drain_inst.ins.sync_info
        waits = list(si.on_wait) if si is not None else []
        if len(waits) > 1:
            si.on_wait = waits[:1]
            drain_inst.ins.sync_info = si
            for w in waits[1:]:
                nop = nc.sync.nop(nofuse=True)
                nop.ins.sync_info = mybir.SyncInfo(on_wait=[w], on_update=[])
        nc.all_engine_barrier()
        popped = nc._tile_sem_poison_stack.pop()
        assert popped is self._sem_poison
        nc.clear_and_free_semaphores(list(self.sems.allocated().values()))
        nc.all_engine_barrier()

    TileContext._drain_and_barrier = _drain_and_barrier
    _PATCHED = True


def _split_multiwaits(nc):
    """This walrus build allows only one sync-wait per instruction: move extra
    waits onto same-engine nops placed immediately before the instruction."""
    n = 0
    for fn in nc.m.functions:
        for blk in fn.blocks:
            newl = []
            for inst in blk.instructions:
                si = inst.sync_info
                if si is not None and len(si.on_wait) > 1:
                    waits = list(si.on_wait)
                    for i, w in enumerate(waits[:-1]):
                        nop = mybir.InstNoOp(
                            name=f"{inst.name}-w{i}", ins=[], outs=[], engine=inst.engine
                        )
                        nop.sync_info = mybir.SyncInfo(on_wait=[w], on_update=[])
                        newl.append(nop)
                        n += 1
                    si.on_wait = waits[-1:]
                    inst.sync_info = si
                newl.append(inst)
            blk.instructions = newl
    return n


def _chunks(S):
    """[(c, p0, pc)] partition chunks covering S: offset p0, size pc<=128."""
    out = []
    c = 0
    while c * P < S:
        out.append((c, c * P, min(P, S - c * P)))
        c += 1
    return out


def build_program(slot_S, L_run=L, taps=(), time_loop=0):
    """Build the SPMD per-core program. slot_S: list of MPC sequence lengths
    (each a multiple of 8, <= 512). taps: debug tensor names to stream out."""
    _patch_drain()
    nc = bass.Bass(trn_type="TRN2", detect_race_conditions=False)

    dram = {}
    taps = set(taps)

    def tap(name, ap, shape=None):
        if name not in taps:
            return
        shape = list(shape if shape is not None else ap.shape)
        t = nc.dram_tensor("dbg_" + name, shape, ap.dtype, kind="ExternalOutput")
        nc.sync.dma_start(out=t[...], in_=ap)

    def din(name, shape, dt):
        dram[name] = nc.dram_tensor(name, list(shape), dt, kind="ExternalInput")
        return dram[name]

    for m, S in enumerate(slot_S):
        KC = len(_chunks(S))
        din(f"mft{m}", (64, S), F16)
        din(f"adjT{m}", (S, S), F16)
        din(f"ednT{m}", (S, S), F16)
        din(f"negc{m}", (KC, 128, 1), F32)
        din(f"poolm{m}", (KC, 128, 1), F16)
    din("wembed", (64, D), F16)
    QKV_DT = F8 if CONFIG["fp8_qkv"] else F16
    WO_DT = F8 if CONFIG["fp8_wo"] else F16
    ffn_mode = CONFIG["fp8_ffn"]
    FFN1_ON = ffn_mode in (True, "ffn1")
    FFN2_ON = ffn_mode in (True, "ffn2")
    FFN1_DT = F8 if FFN1_ON else F16
    FFN2_DT = F8 if FFN2_ON else F16
    din("wq", (L, 2, D, DH), QKV_DT)
    din("wk", (L, 2, D, DH), QKV_DT)
    din("wv", (L, D, 2 * DH), QKV_DT)
    din("wo", (L, D, D), WO_DT)
    din("wf1", (L, D, DFF), FFN1_DT)
    din("wf2", (L, DFF, D), FFN2_DT)
    din("wh1", (C, D, U1), F16)
    din("wh2", (C, U1, U2), F16)
    din("chain", (C, MPC, U2), F32)
    din("ident32", (128, 128), F32)
    din("ident16", (128, 128), F16)
    out_dram = nc.dram_tensor("out", [C, MPC, U2], F32, kind="ExternalOutput")

    MOLS = list(range(len(slot_S)))

    with TileContext(nc) as tc:
        with ExitStack() as ctx:
            pers = ctx.enter_context(tc.tile_pool(name="pers", bufs=1))
            wpool = ctx.enter_context(tc.tile_pool(name="wpool", bufs=2))
            wkq = ctx.enter_context(tc.tile_pool(name="wkq", bufs=1))
            wk2 = ctx.enter_context(tc.tile_pool(name="work", bufs=CONFIG.get("wk2_bufs", 2)))
            wk3 = ctx.enter_context(tc.tile_pool(name="work3", bufs=CONFIG.get("wk3_bufs", 6)))
            # PSUM budget is 8 banks total
            ps_s = ctx.enter_context(
                tc.tile_pool(name="ps_s", bufs=CONFIG["ps_s_bufs"], space="PSUM")
            )
            if CONFIG["ps_split"]:
                ps_o = ctx.enter_context(
                    tc.tile_pool(name="ps_o", bufs=CONFIG["ps_o_bufs"], space="PSUM")
                )
            ps_a = ctx.enter_context(
                tc.tile_pool(name="ps_a", bufs=CONFIG["ps_a_bufs"], space="PSUM")
            )
            if not CONFIG["ps_split"]:
                ps_o = ps_a

            # ---- persistent constants / state ----
            ident_32 = pers.tile([128, 128], F32, tag="id32")
            nc.sync.dma_start(out=ident_32, in_=dram["ident32"][:, :])
            ident_16 = pers.tile([128, 128], F16, tag="id16")
            nc.sync.dma_start(out=ident_16, in_=dram["ident16"][:, :])
            ones16 = pers.tile([128, 32], F16, tag="ones16")
            nc.vector.memset(ones16, 1.0)
            eps_col = pers.tile([128, 1], F32, tag="eps")
            nc.vector.memset(eps_col, EPS_LN)
            eps12_col = pers.tile([128, 1], F32, tag="eps12")
            nc.vector.memset(eps12_col, 1.0e-12)


            negc, poolm, xT, xtok, adjT, ednT, mft = [], [], [], [], [], [], []
            for m, S in enumerate(slot_S):
                ch = _chunks(S)
                KC = len(ch)
                t = pers.tile([128, KC, 1], F32, tag=f"negc{m}", name=f"negc{m}")
                nc.sync.dma_start(
                    out=t, in_=dram[f"negc{m}"].rearrange("c p one -> p c one")
                )
                negc.append(t)
                t = pers.tile([128, KC, 1], F16, tag=f"poolm{m}", name=f"poolm{m}")
                nc.sync.dma_start(
                    out=t, in_=dram[f"poolm{m}"].rearrange("c p one -> p c one")
                )
                poolm.append(t)
                xT.append(
                    pers.tile(
                        [128, 2, 512 if CONFIG["fp8_qkv"] else S],
                        QKV_DT,
                        tag=f"xT{m}",
                        name=f"xT{m}",
                    )
                )
                xtok.append(pers.tile([128, KC, D], F16, tag=f"xtok{m}", name=f"xtok{m}"))
                t = pers.tile([128, KC, S], F16, tag=f"adjT{m}", name=f"adjT{m}")
                for c, p0, pc in ch:
                    nc.sync.dma_start(
                        out=t[0:pc, c, :], in_=dram[f"adjT{m}"][p0 : p0 + pc, :]
                    )
                adjT.append(t)
                t = pers.tile([128, KC, S], F16, tag=f"ednT{m}", name=f"ednT{m}")
                for c, p0, pc in ch:
                    nc.sync.dma_start(
                        out=t[0:pc, c, :], in_=dram[f"ednT{m}"][p0 : p0 + pc, :]
                    )
                ednT.append(t)
                t = pers.tile([128, S], F16, tag=f"mft{m}", name=f"mft{m}")
                nc.sync.dma_start(out=t[0:64, :], in_=dram[f"mft{m}"][:, :])
                mft.append(t)

            wh1_sb = pers.tile([128, C, 2, U1], F16, tag="wh1")
            nc.sync.dma_start(
                out=wh1_sb, in_=dram["wh1"].rearrange("k (c p) n -> p k c n", p=128)
            )
            wh2_sb = pers.tile([128, C, 4, U2], F16, tag="wh2")
            nc.sync.dma_start(
                out=wh2_sb, in_=dram["wh2"].rearrange("k (c p) n -> p k c n", p=128)
            )
            wembed_sb = pers.tile([128, D], F16, tag="wembed")
            nc.sync.dma_start(out=wembed_sb[0:64, :], in_=dram["wembed"][:, :])

            prow = pers.tile([128, MPC * D], F32, tag="prow")
            MM = nc.tensor.matmul

            # per-mol tiles that live across layer steps
            qkT = [
                wkq.tile([128, 4, S], F16, tag=f"qkT{m}", name=f"qkT{m}")
                for m, S in enumerate(slot_S)
            ]
            Vt = [
                wkq.tile([128, KC_, 2 * DH], F16, tag=f"Vt{m}", name=f"Vt{m}")
                for m, (S, KC_) in enumerate(
                    (S, len(_chunks(S))) for S in slot_S
                )
            ]
            osc = [
                wkq.tile(
                    [128, 2, 512 if CONFIG["fp8_wo"] else S],
                    WO_DT,
                    tag=f"osc{m}",
                    name=f"osc{m}",
                )
                for m, S in enumerate(slot_S)
            ]

            from contextlib import nullcontext

            with (tc.For_i(0, time_loop, 1) if time_loop else nullcontext()):
                # ---- phase 0: embed ----
                for m, S in enumerate(slot_S):
                    ch = _chunks(S)
                    for c2 in range(2):
                        pse = ps_a.tile([128, 512], F32, tag="a")
                        MM(
                            pse[:, 0:S],
                            wembed_sb[0:64, c2 * 128 : (c2 + 1) * 128],
                            mft[m][0:64, :],
                            start=True,
                            stop=True,
                        )
                        nc.scalar.copy(out=xT[m][:, c2, 0:S], in_=pse[:, 0:S])
                    for c, p0, pc in ch:
                        pse = ps_a.tile([128, 512], F32, tag="a")
                        MM(
                            pse[0:pc, 0:D],
                            mft[m][0:64, p0 : p0 + pc],
                            wembed_sb[0:64, :],
                            start=True,
                            stop=True,
                        )
                        nc.vector.tensor_copy(out=xtok[m][0:pc, c, :], in_=pse[0:pc, 0:D])

                tap("xT0", xT[0][:, :, :])
                tap("xtok0", xtok[0][:, :, :])

                # ---- layers ----
                for l in range(L_run):
                    wq_sb = wpool.tile([128, 2, 2, DH], QKV_DT, tag="wq")
                    nc.sync.dma_start(
                        out=wq_sb,
                        in_=dram["wq"][l].rearrange("b (c p) n -> p b c n", p=128),
                    )
                    wk_sb = wpool.tile([128, 2, 2, DH], QKV_DT, tag="wk")
                    nc.sync.dma_start(
                        out=wk_sb,
                        in_=dram["wk"][l].rearrange("b (c p) n -> p b c n", p=128),
                    )
                    wv_sb = wpool.tile([128, 2, 2 * DH], QKV_DT, tag="wv")
                    nc.sync.dma_start(
                        out=wv_sb, in_=dram["wv"][l].rearrange("(c p) n -> p c n", p=128)
                    )
                    wo_sb = wpool.tile([128, 2, D], WO_DT, tag="wo")
                    nc.sync.dma_start(
                        out=wo_sb, in_=dram["wo"][l].rearrange("(c p) n -> p c n", p=128)
                    )
                    wf1_sb = wpool.tile([128, 2, DFF], FFN1_DT, tag="wf1")
                    nc.sync.dma_start(
                        out=wf1_sb, in_=dram["wf1"][l].rearrange("(c p) n -> p c n", p=128)
                    )
                    wf2_sb = wpool.tile([128, 8, D], FFN2_DT, tag="wf2")
                    nc.sync.dma_start(
                        out=wf2_sb, in_=dram["wf2"][l].rearrange("(c p) n -> p c n", p=128)
                    )

                    # ---- S1+S2: q/k/v projections for mol m ----
                    def proj_qkv(m):
                        S = slot_S[m]
                        ch = _chunks(S)
                        for br in range(2):
                            for which, w_sb in ((0, wq_sb), (1, wk_sb)):
                                pp = ps_a.tile([128, 512], F32, tag="a")
                                if CONFIG["fp8_qkv"]:
                                    MM(
                                        pp[:, 0:S],
                                        w_sb[:, br, :, :],
                                        xT[m][:, :, 0:S],
                                        start=True,
                                        stop=True,
                                        perf_mode=DR,
                                    )
                                else:
                                    for cc in range(2):
                                        MM(
                                            pp[:, 0:S],
                                            w_sb[:, br, cc, :],
                                            xT[m][:, cc, 0:S],
                                            start=(cc == 0),
                                            stop=(cc == 1),
                                        )
                                # qkT slots: 0,1 = qT br0/br1; 2,3 = kT br0/br1
                                slot = which * 2 + br
                                qs = (1.0 / WS) if CONFIG["fp8_qkv"] else 1.0
                                if which == 0:
                                    nc.scalar.activation(
                                        qkT[m][:, slot, :],
                                        pp[:, 0:S],
                                        AF.Copy,
                                        scale=qs,
                                    )
                                else:
                                    nc.vector.tensor_scalar(
                                        out=qkT[m][:, slot, :],
                                        in0=pp[:, 0:S],
                                        scalar1=qs,
                                        scalar2=None,
                                        op0=ALU.mult,
                                    )
                        for c, p0, pc in ch:
                            pv = ps_a.tile([128, 512], F32, tag="a")
                            if CONFIG["fp8_qkv"]:
                                MM(
                                    pv[0:pc, 0 : 2 * DH],
                                    xT[m][:, :, p0 : p0 + pc],
                                    wv_sb[:, :, :],
                                    start=True,
                                    stop=True,
                                    perf_mode=DR,
                                )
                            else:
                                for cc in range(2):
                                    MM(
                                        pv[0:pc, 0 : 2 * DH],
                                        xT[m][:, cc, p0 : p0 + pc],
                                        wv_sb[:, cc, :],
                                        start=(cc == 0),
                                        stop=(cc == 1),
                                    )
                            nc.vector.tensor_scalar(
                                out=Vt[m][0:pc, c, :],
                                in0=pv[0:pc, 0 : 2 * DH],
                                scalar1=(1.0 / WS) if CONFIG["fp8_qkv"] else 1.0,
                                scalar2=None,
                                op0=ALU.mult,
                            )

                    # ---- S3: attention both branches for mol m ----
                    def attn(m):
                        S = slot_S[m]
                        ch = _chunks(S)
                        KC = len(ch)
                        otag = "o" if CONFIG["ps_split"] else "a"
                        PTs, Aws = {}, {}

                        def attn_p1(br, c, p0, pc):
                            qT = qkT[m][:, 0 + br, :]
                            kT = qkT[m][:, 2 + br, :]
                            MT = ednT[m] if br == 0 else adjT[m]
                            PT = wk3.tile([128, 4, S], F16, tag="PT", bufs=CONFIG.get("pt_bufs", 6))
                            for pair in range(2):
                                ps = ps_s.tile([128, 2, 512], F32, tag="s")
                                for i in range(2):
                                    h = 2 * pair + i
                                    MM(
                                        ps[0:pc, i, 0:S],
                                        kT[32 * h : 32 * h + 32, p0 : p0 + pc],
                                        qT[32 * h : 32 * h + 32, :],
                                        start=True,
                                        stop=True,
                                        tile_position=(32 * h, 0),
                                    )
                                nc.scalar.activation(
                                    PT[0:pc, 2 * pair : 2 * pair + 2, :],
                                    ps[0:pc, :, 0:S],
                                    AF.Exp,
                                    bias=negc[m][0:pc, c, :],
                                    scale=1.0,
                                )
                            Aw = wk3.tile([128, 4, S], F16, tag="Aw", bufs=CONFIG.get("pt_bufs", 6))
                            nc.vector.tensor_mul(
                                Aw[0:pc, :, :],
                                PT[0:pc, :, :],
                                MT[0:pc, c : c + 1, :].broadcast_to((pc, 4, S)),
                            )
                            PTs[(br, c)] = PT
                            Aws[(br, c)] = Aw

                        def attn_p2(br):
                            po = ps_o.tile([128, 512], F32, tag=otag)
                            pr = ps_o.tile([128, 512], F32, tag=otag)
                            for c, p0, pc in ch:
                                for h in range(4):
                                    MM(
                                        pr[32 * h : 32 * h + 32, 0:S],
                                        ones16[0:pc, 0:32],
                                        PTs[(br, c)][0:pc, h, :],
                                        start=(c == 0),
                                        stop=(c == KC - 1),
                                        tile_position=(0, 32 * h),
                                        skip_group_check=True,
                                    )
                                    MM(
                                        po[32 * h : 32 * h + 32, 0:S],
                                        Vt[m][
                                            0:pc,
                                            c,
                                            br * DH + 32 * h : br * DH + 32 * h + 32,
                                        ],
                                        Aws[(br, c)][0:pc, h, :],
                                        start=(c == 0),
                                        stop=(c == KC - 1),
                                        tile_position=(0, 32 * h),
                                        skip_group_check=True,
                                    )
                            recipb = wk2.tile([128, S], F32, tag="recipb")
                            nc.vector.reciprocal(out=recipb[:, :], in_=pr[:, 0:S])
                            nc.vector.scalar_tensor_tensor(
                                out=osc[m][:, br, 0:S],
                                in0=po[:, 0:S],
                                scalar=QS if CONFIG["fp8_wo"] else 1.0,
                                in1=recipb[:, :],
                                op0=ALU.mult,
                                op1=ALU.mult,
                            )

                        if CONFIG["br_inter"]:
                            for c, p0, pc in ch:
                                attn_p1(0, c, p0, pc)
                                attn_p1(1, c, p0, pc)
                            attn_p2(0)
                            attn_p2(1)
                        else:
                            for br in range(2):
                                for c, p0, pc in ch:
                                    attn_p1(br, c, p0, pc)
                                attn_p2(br)
                        if l == 0 and m == 0:
                            tap("osc0", osc[m][:, :, :])

                    # ---- S4+S5+S6: Wo+LN1, FFN1, FFN2+LN2, next xT for mol m ----
                    def ffn_block(m):
                        S = slot_S[m]
                        ch = _chunks(S)
                        KC = len(ch)
                        # Wo + residual + LN1
                        xres = wk2.tile([128, KC, D], F16, tag="xres")
                        xln1 = wk3.tile([128, KC, D], F16, tag="xln1")
                        mv = wk2.tile([128, KC, 2], F32, tag="mv")
                        if S % 128:
                            pal = ((S % 128) // 32) * 32
                            nc.vector.memset(xres[pal:128, KC - 1, :], 0.0)
                        for c, p0, pc in ch:
                            pw = ps_a.tile([128, 512], F32, tag="a")
                            if CONFIG["fp8_wo"]:
                                MM(
                                    pw[0:pc, 0:D],
                                    osc[m][:, :, p0 : p0 + pc],
                                    wo_sb[:, :, :],
                                    start=True,
                                    stop=True,
                                    perf_mode=DR,
                                )
                            else:
                                for cc in range(2):
                                    MM(
                                        pw[0:pc, 0:D],
                                        osc[m][:, cc, p0 : p0 + pc],
                                        wo_sb[:, cc, :],
                                        start=(cc == 0),
                                        stop=(cc == 1),
                                    )
                            nc.vector.scalar_tensor_tensor(
                                out=xres[0:pc, c, :],
                                in0=pw[0:pc, 0:D],
                                scalar=(1.0 / (WS * QS)) if CONFIG["fp8_wo"] else 1.0,
                                in1=xtok[m][0:pc, c, :],
                                op0=ALU.mult,
                                op1=ALU.add,
                            )
                        bst = wk3.tile([128, KC, 6], F32, tag="bst")
                        for c in range(KC):
                            nc.vector.bn_stats(out=bst[:, c, :], in_=xres[:, c, :])
                            nc.vector.bn_aggr(out=mv[:, c, :], in_=bst[:, c, :])
                        if l == 0 and m == 0:
                            tap("xres0", xres[:, :, :])
                        # rstd = exp(-0.5*ln(var+eps)) - stays in the exp table
                        sd = wk2.tile([128, KC], F32, tag="sd")
                        nc.scalar.activation(
                            sd[:, :], mv[:, :, 1], AF.Ln, bias=eps_col[:, :], scale=1.0
                        )
                        rstd = wk2.tile([128, KC], F32, tag="rstd")
                        nc.scalar.activation(rstd[:, :], sd[:, :], AF.Exp, scale=-0.5)
                        for c, p0, pc in ch:
                            nc.vector.tensor_scalar(
                                out=xln1[0:pc, c, :],
                                in0=xres[0:pc, c, :],
                                scalar1=mv[0:pc, c, 0:1],
                                scalar2=rstd[0:pc, c : c + 1],
                                op0=ALU.subtract,
                                op1=ALU.mult,
                            )
                        if l == 0 and m == 0:
                            tap("xln1_0", xln1[:, :, :])
                        # x_ln1^T (feature-major fp8) via PE transpose (f16 in)
                        xln1T = wk2.tile([128, 2, 512], FFN1_DT, tag="xln1T")
                        for cc in range(2):
                            pt = ps_a.tile([128, 512], F16, tag="a")
                            for c, p0, pc in ch:
                                nc.tensor.transpose(
                                    pt[:, p0 : p0 + pc],
                                    xln1[0:pc, c, cc * 128 : (cc + 1) * 128],
                                    ident_16[0:pc, 0:pc],
                                )
                            nc.scalar.copy(out=xln1T[:, cc, 0:S], in_=pt[:, 0:S])
                        if l == 0 and m == 0:
                            tap("xln1T0", xln1T[:, :, :])
                        # FFN1 (feature-major) with relu -> h^T fp8
                        hT = wk2.tile([128, 8, 512 if FFN2_ON else S], FFN2_DT, tag="hT")
                        for hc in range(8):
                            pf = ps_a.tile([128, 512], F32, tag="a")
                            if FFN1_ON:
                                MM(
                                    pf[:, 0:S],
                                    wf1_sb[:, :, hc * 128 : (hc + 1) * 128],
                                    xln1T[:, :, 0:S],
                                    start=True,
                                    stop=True,
                                    perf_mode=DR,
                                )
                            else:
                                for cc in range(2):
                                    MM(
                                        pf[:, 0:S],
                                        wf1_sb[:, cc, hc * 128 : (hc + 1) * 128],
                                        xln1T[:, cc, 0:S],
                                        start=(cc == 0),
                                        stop=(cc == 1),
                                    )
                            hs = (1.0 / WS) if FFN1_ON else 1.0
                            if hc % 2 == 0:
                                nc.scalar.activation(
                                    hT[:, hc, 0:S], pf[:, 0:S], AF.Relu, scale=hs
                                )
                            else:
                                nc.vector.tensor_scalar(
                                    out=hT[:, hc, 0:S],
                                    in0=pf[:, 0:S],
                                    scalar1=0.0,
                                    scalar2=hs,
                                    op0=ALU.max,
                                    op1=ALU.mult,
                                )
                        if l == 0 and m == 0:
                            tap("hT0", hT[:, :, :])
                        # FFN2 tokens-major: stationary = hT chunk, moving = wf2
                        mv2 = wk2.tile([128, KC, 2], F32, tag="mv2")
                        xres2 = wk2.tile([128, KC, D], F16, tag="xres2")
                        if S % 128:
                            pal = ((S % 128) // 32) * 32
                            nc.vector.memset(xres2[pal:128, KC - 1, :], 0.0)
                        for c, p0, pc in ch:
                            pf = ps_a.tile([128, 512], F32, tag="a")
                            if FFN2_ON:
                                for j in range(4):
                                    MM(
                                        pf[0:pc, 0:D],
                                        hT[:, 2 * j : 2 * j + 2, p0 : p0 + pc],
                                        wf2_sb[:, 2 * j : 2 * j + 2, :],
                                        start=(j == 0),
                                        stop=(j == 3),
                                        perf_mode=DR,
                                    )
                            else:
                                for hc in range(8):
                                    MM(
                                        pf[0:pc, 0:D],
                                        hT[:, hc, p0 : p0 + pc],
                                        wf2_sb[:, hc, :],
                                        start=(hc == 0),
                                        stop=(hc == 7),
                                    )
                            nc.vector.scalar_tensor_tensor(
                                out=xres2[0:pc, c, :],
                                in0=pf[0:pc, 0:D],
                                scalar=(1.0 / WS) if FFN2_ON else 1.0,
                                in1=xln1[0:pc, c, :],
                                op0=ALU.mult,
                                op1=ALU.add,
                            )
                        bst = wk3.tile([128, KC, 6], F32, tag="bst")
                        for c in range(KC):
                            nc.vector.bn_stats(out=bst[:, c, :], in_=xres2[:, c, :])
                            nc.vector.bn_aggr(out=mv2[:, c, :], in_=bst[:, c, :])
                        sd2 = wk2.tile([128, KC], F32, tag="sd2")
                        nc.scalar.activation(
                            sd2[:, :], mv2[:, :, 1], AF.Ln, bias=eps_col[:, :], scale=1.0
                        )
                        rstd2 = wk2.tile([128, KC], F32, tag="rstd2")
                        nc.scalar.activation(rstd2[:, :], sd2[:, :], AF.Exp, scale=-0.5)
                        for c, p0, pc in ch:
                            nc.vector.tensor_scalar(
                                out=xtok[m][0:pc, c, :],
                                in0=xres2[0:pc, c, :],
                                scalar1=mv2[0:pc, c, 0:1],
                                scalar2=rstd2[0:pc, c : c + 1],
                                op0=ALU.subtract,
                                op1=ALU.mult,
                            )
                        if l == 0 and m == 0:
                            tap("xtok_l0", xtok[m][:, :, :])
                        if l < L_run - 1:
                            for cc in range(2):
                                pt = ps_a.tile([128, 512], F16, tag="a")
                                for c, p0, pc in ch:
                                    nc.tensor.transpose(
                                        pt[:, p0 : p0 + pc],
                                        xtok[m][0:pc, c, cc * 128 : (cc + 1) * 128],
                                        ident_16[0:pc, 0:pc],
                                    )
                                nc.scalar.copy(out=xT[m][:, cc, 0:S], in_=pt[:, 0:S])

                    def pool_mol(m):
                        S = slot_S[m]
                        ch = _chunks(S)
                        pp = ps_a.tile([128, 512], F32, tag="a")
                        for c, p0, pc in ch:
                            MM(
                                pp[0:1, 0:D],
                                poolm[m][0:pc, c, :],
                                xtok[m][0:pc, c, :],
                                start=(c == 0),
                                stop=(c == len(ch) - 1),
                            )
                        nc.vector.tensor_copy(
                            out=prow[0:1, m * D : (m + 1) * D], in_=pp[0:1, 0:D]
                        )

                    # skewed emission: proj runs ahead of attn, attn ahead of
                    # the ffn block, so each engine sees independent work from
                    # neighbouring molecules between dependent stages
                    stages = [proj_qkv, attn, ffn_block]
                    offs = list(CONFIG["skew"])
                    if l == L_run - 1:
                        stages.append(pool_mol)
                        offs.append(offs[-1] + 1)
                    for t in range(len(MOLS) + max(offs)):
                        for si in range(len(stages)):
                            mi = t - offs[si]
                            if 0 <= mi < len(MOLS):
                                stages[si](mi)

                    if l == 0:
                        tap("qkT0", qkT[0][:, :, :])
                        tap("Vt0", Vt[0][:, :, :])

                # ---- contrastive heads (pooling ran as a last-layer stage) ----
                ones32c = pers.tile([128, 1], F32, tag="ones32c")
                nc.vector.memset(ones32c, 1.0)
                # pooled^T (256 x MPC) fp16 via K=1 matmuls off the partition-0 row
                pT_sb = pers.tile([128, 2, MPC], F16, tag="pT")
                ptp = ps_a.tile([128, 512], F32, tag="a")
                for cc in range(2):
                    for m in range(MPC):
                        MM(
                            ptp[:, cc * MPC + m : cc * MPC + m + 1],
                            prow[0:1, m * D + cc * 128 : m * D + (cc + 1) * 128],
                            ones32c[0:1, 0:1],
                            start=(cc == 0 and m == 0),
                            stop=(cc == 1 and m == MPC - 1),
                            skip_group_check=True,
                        )
                nc.vector.tensor_copy(
                    out=pT_sb[:, :, :],
                    in_=ptp[:, 0 : 2 * MPC].rearrange("p (c m) -> p c m", c=2),
                )
                for k in range(C):
                    h1 = wk2.tile([128, 4, MPC], F16, tag="h1")
                    for u in range(4):
                        ph = ps_a.tile([128, 512], F32, tag="a")
                        for cc in range(2):
                            MM(
                                ph[:, 0:MPC],
                                wh1_sb[:, k, cc, u * 128 : (u + 1) * 128],
                                pT_sb[:, cc, :],
                                start=(cc == 0),
                                stop=(cc == 1),
                            )
                        nc.scalar.activation(h1[:, u, :], ph[:, 0:MPC], AF.Relu)
                    h2m = wk2.tile([128, D], F32, tag="h2m")
                    ph2 = ps_a.tile([128, 512], F32, tag="a")
                    for oc in range(2):
                        for uc in range(4):
                            MM(
                                ph2[:, oc * MPC : (oc + 1) * MPC],
                                wh2_sb[:, k, uc, oc * 128 : (oc + 1) * 128],
                                h1[:, uc, :],
                                start=(uc == 0),
                                stop=(uc == 3),
                            )
                    # relu into fp16 tile, transpose to (MPC x 256) rows
                    h2f = wk2.tile([128, 2, MPC], F16, tag="h2f")
                    for oc in range(2):
                        nc.scalar.activation(
                            h2f[:, oc, :], ph2[:, oc * MPC : (oc + 1) * MPC], AF.Relu
                        )
                    pht = ps_a.tile([128, 512], F16, tag="a")
                    for oc in range(2):
                        nc.tensor.transpose(
                            pht[0:MPC, oc * 128 : (oc + 1) * 128],
                            h2f[:, oc, :],
                            ident_16[:, :],
                        )
                    nc.vector.tensor_copy(out=h2m[0:MPC, :], in_=pht[0:MPC, 0:D])
                    # l2 normalize rows: rs = exp(-0.5*ln(ss+1e-12))
                    sq = wk2.tile([128, D], F32, tag="sq")
                    nc.vector.tensor_mul(sq[0:MPC, :], h2m[0:MPC, :], h2m[0:MPC, :])
                    ss = wk2.tile([128, 1], F32, tag="ss")
                    nc.vector.reduce_sum(
                        ss[0:MPC, :], sq[0:MPC, :], axis=mybir.AxisListType.X
                    )
                    lt = wk2.tile([128, 1], F32, tag="lt")
                    nc.scalar.activation(
                        lt[0:MPC, :], ss[0:MPC, :], AF.Ln, bias=eps12_col[0:MPC, :]
                    )
                    rs = wk2.tile([128, 1], F32, tag="rs")
                    nc.scalar.activation(rs[0:MPC, :], lt[0:MPC, :], AF.Exp, scale=-0.5)
                    fin = wk2.tile([128, D], F32, tag="fin")
                    nc.vector.tensor_scalar(
                        out=fin[0:MPC, :],
                        in0=h2m[0:MPC, :],
                        scalar1=rs[0:MPC, :],
                        scalar2=None,
                        op0=ALU.mult,
                    )
                    nc.sync.dma_start(out=out_dram[k], in_=fin[0:MPC, :])

    _split_multiwaits(nc)
    return nc


# ----------------------------------------------------------------------------
# host side
# ----------------------------------------------------------------------------


def _prep_weights(inputs):
    f16 = np.float16
    f8 = mybir.dt.np(F8)

    def q8(w, on):
        if not on:
            return np.asarray(w, np.float32).astype(f16)
        return np.clip(np.asarray(w, np.float32) * WS, -224.0, 224.0).astype(f8)

    wq = np.stack(
        [inputs["W_qkv"][:, 0] / np.sqrt(DEPTH), inputs["W_qkv"][:, 3] / np.sqrt(DEPTH)],
        axis=1,
    )
    wk = np.stack([inputs["W_qkv"][:, 1], inputs["W_qkv"][:, 4]], axis=1)
    wv = np.concatenate([inputs["W_qkv"][:, 2], inputs["W_qkv"][:, 5]], axis=-1)
    return {
        "chain": np.zeros((C, MPC, U2), np.float32),
        "wembed": inputs["W_embed"].astype(f16),
        "wq": q8(wq, CONFIG["fp8_qkv"]),
        "wk": q8(wk, CONFIG["fp8_qkv"]),
        "wv": q8(wv, CONFIG["fp8_qkv"]),
        "wo": q8(inputs["W_o"], CONFIG["fp8_wo"]),
        "wf1": q8(inputs["W_ff1"], CONFIG["fp8_ffn"] in (True, "ffn1")),
        "wf2": q8(inputs["W_ff2"], CONFIG["fp8_ffn"] in (True, "ffn2")),
        "wh1": inputs["Wh1"].astype(f16),
        "wh2": inputs["Wh2"].astype(f16),
        "ident32": np.eye(128, dtype=np.float32),
        "ident16": np.eye(128, dtype=np.float16),
    }


def _check_trivial(inputs):
    z = [
        "b_embed",
        "b_qkv",
        "b_o",
        "b_ff1",
        "b_ff2",
        "bh1",
        "bh2",
        "ln1_b",
        "ln2_b",
    ]
    ok = all(np.abs(inputs[k]).max() == 0.0 for k in z)
    ok = ok and np.all(inputs["ln1_g"] == 1.0) and np.all(inputs["ln2_g"] == 1.0)
    if not ok:
        raise NotImplementedError(
            "kernel specialized for zero biases / unit layernorm gains (per spec)"
        )


def _mol_arrays(b_idx, inputs, S, perm=None):
    """Per-molecule prepped arrays, optionally token-permuted, truncated to S."""
    mol = np.asarray(inputs["mol_feat"][b_idx])
    adj = np.asarray(inputs["adj"][b_idx])
    dist = np.asarray(inputs["dist"][b_idx])
    mask = np.asarray(inputs["mask"][b_idx, 0, 0, :])
    if perm is not None:
        mol, adj, dist, mask = (
            mol[perm],
            adj[perm][:, perm],
            dist[perm][:, perm],
            mask[perm],
        )
    mol, adj, dist, mask = mol[:S], adj[:S, :S], dist[:S, :S], mask[:S]
    KC = len(_chunks(S))
    negc = np.full((KC, 128, 1), NEG, np.float32)
    poolm = np.zeros((KC, 128, 1), np.float16)
    negflat = (mask * NEG).astype(np.float32)
    poolflat = (mask == 0).astype(np.float32)
    for c, p0, pc in _chunks(S):
        negc[c, 0:pc, 0] = negflat[p0 : p0 + pc]
        poolm[c, 0:pc, 0] = poolflat[p0 : p0 + pc]
    # host-side softmax(dist + neg) over keys (layer-invariant)
    e = np.exp(dist.astype(np.float64)) * (mask == 0)[None, :]
    edn = e / e.sum(axis=1, keepdims=True)
    return {
        "mft": np.ascontiguousarray(mol.T).astype(np.float16),
        "adjT": np.ascontiguousarray(adj.T).astype(np.float16),
        "ednT": np.ascontiguousarray(edn.T).astype(np.float16),
        "negc": negc,
        "poolm": poolm,
    }


def plan(inputs):
    """Sort molecules by real-token count; slot s of every core gets one of the
    8 molecules ranked [8s, 8s+8); slot length = roundup8(max real in group)."""
    mask = np.asarray(inputs["mask"])[:, 0, 0, :]
    real = (mask == 0).sum(1)
    order = np.argsort(-real, kind="stable")
    slot_S, assign = [], [[0] * MPC for _ in range(NCORES)]
    for s in range(MPC):
        group = order[NCORES * s : NCORES * (s + 1)]
        Smax = int(min(((int(real[group].max()) + 7) // 8) * 8, S_FULL))
        slot_S.append(Smax)
        for c in range(NCORES):
            assign[c][s] = int(group[c])
    return slot_S, assign


def make_in_maps(inputs, slot_S, assign, compact=True):
    """assign[c][m] = molecule index for core c, slot m."""
    w = _prep_weights(inputs)
    mask_all = np.asarray(inputs["mask"])[:, 0, 0, :]
    in_maps = []
    for c in range(NCORES):
        im = dict(w)
        for m in range(MPC):
            b = assign[c][m]
            perm = None
            if compact:
                perm = np.argsort(mask_all[b], kind="stable")
            arrs = _mol_arrays(b, inputs, slot_S[m], perm=perm)
            for k, v in arrs.items():
                im[f"{k}{m}"] = v
        in_maps.append(im)
    return in_maps


def kernel(**inputs):
    _check_trivial(inputs)
    slot_S, assign = plan(inputs)
    nc = build_program(slot_S)
    in_maps = make_in_maps(inputs, slot_S, assign)
    from concourse.bass_utils import run_bass_kernel_spmd

    res = run_bass_kernel_spmd(nc, in_maps, core_ids=list(range(NCORES)))
    out = np.zeros((C, B, U2), np.float32)
    for c in range(NCORES):
        o = res.results[c]["out"]  # (C, MPC, U2)
        for m in range(MPC):
            out[:, assign[c][m], :] = o[:, m, :]
    return out
